# revision 24
# baseline (speedup 1.0000x reference)
"""Trainium2 Bass kernel for a dense transformer block (B=128, T=256, C=384,
H=6, HD=64, FFN=4C), data-parallel over batch across 8 NeuronCores.

Layout strategy (per core, 16 batch elements):
  - LayerNorm runs token-major ([128 tokens, 384] tiles, per-partition stats).
  - Matmul inputs are fp16 (PE runs 1 cycle/row at any free dim, FWL weight
    loads); accumulation is fp32 in PSUM; the residual stream stays fp32.
  - x1/x2/attn are transposed to feature-major with PE identity-matmuls so
    every matmul uses naturally-laid-out operands.
  - Softmax is computed s-major (scoresT = k_f^T q_f per head), exp on the
    scalar engine, causal mask applied by gpsimd affine_select (zero fill).
  - Attention output is computed token-major with the softmax denominator
    folded in as a 65th "ones" column of v; normalization is then a cheap
    per-partition reciprocal + broadcast multiply during PSUM evacuation.
  - LN gains are folded into the adjacent weight matrices host-side; biases
    (all zero in this problem) have exact fallback paths emitted only when
    nonzero at build time.
  - Scores (K=64 per head) pack head pairs into disjoint PE row groups via
    tile_position (0,0)/(64,0): the two matmuls run concurrently in the
    128x128 array, halving score time (invisible to CoreSim, real on HW).
  - fl5_head_ilv threads single-mp FFN1 filler matmuls between the i=0 attn
    head-pairs so the N=65 attn matmuls' LDWEIGHTS stream behind real work.
    Finer-grained filler variants (between i=1 pairs, inside transpose
    groups) were measured SLOWER on HW - the transpose->evac->consumer
    chains are latency-critical; don't put fillers inside them.
  - fp8e4 DoubleRow was evaluated and rejected: one fp8 GEMM alone costs
    ~2.2e-2 max-rel-err vs the 2e-2 gate (numpy study in fp8_study.py).
"""

import sys

sys.path.insert(0, "/opt/trn_rl_repo")

import numpy as np

import concourse.bass as bass
import concourse.tile as tile
from concourse import bacc, mybir
from concourse.bass_utils import run_bass_kernel_spmd

FP32 = mybir.dt.float32
FP16 = mybir.dt.float16
AF = mybir.ActivationFunctionType
ALU = mybir.AluOpType

N_CORES = 8
B, T, C, H, HD = 128, 256, 384, 6, 64
F = 4 * C  # 1536
BC = B // N_CORES  # 16 batches per core
NT = T // 128  # 2 token chunks per batch
NC_ = C // 128  # 3 feature chunks
NF = F // 128  # 12 hidden chunks
EPS = 1e-5
SCALE = HD ** -0.5

_PROGRAM_CACHE = {}
POOL_CFG = dict(apool=5, psA=6, psT=2, spool=8, lookahead=0, order="FLAG5",
                fl5_m4_early=True, fl3_relu_dve=1, fl5_f1_thirds=True,
                fl5_attn_split=True, fl5_splits=(2, 3, 5), fl5_f2_pos=2,
                fl5_head_ilv=True,
                rstd_pow=False, ln_apply_gpsimd=False, v_evac_scalar=False,
                head_pair=False, relu_dve=3, tr_evac_scalar="", qk_evac_dve=False,
                w384=True, fine_attn=False, attn_evac_act=False,
                xpool=3, xla=2, qk_merge=True, attn_inline=True,
                res16=False, attn_evac_act2=False,
                tsplit=True, qk_nosplit=True, ffn1_split=True,
                score_rowtile=True)


def soff_legacy(i, si):
    return {(0, 0): 0, (1, 0): 128, (1, 1): 256}[(i, si)]


def _patched_act_tables(arch):
    """Force every activation onto the one table set that contains all the
    functions this kernel uses (ln, exp, relu, copy, identity), so the ACT
    engine loads its spline tables exactly once instead of thrashing between
    per-function default sets (~1.3us per reload)."""
    import concourse.hw_specs as hw_specs
    full = hw_specs.get_activation_tables(arch)
    target = "natural_log_exp_and_others"
    return {k: (v if k == target else set()) for k, v in full.items()}


def build_program(flags, repeat=1):
    """flags: (use_qb, use_kb, use_vb, use_bo, use_b2, use_b1) booleans."""
    use_qb, use_kb, use_vb, use_bo, use_b2, use_b1 = flags
    bacc.get_activation_tables = _patched_act_tables
    nc = bacc.Bacc("TRN2", target_bir_lowering=False, debug=False,
                   num_devices=N_CORES)

    x_d = nc.dram_tensor("x", [BC, T, C], FP32, kind="ExternalInput").ap()
    wq_d = nc.dram_tensor("wq", [C, C], FP32, kind="ExternalInput").ap()
    wk_d = nc.dram_tensor("wk", [C, C], FP32, kind="ExternalInput").ap()
    wv_d = nc.dram_tensor("wv", [C, C], FP32, kind="ExternalInput").ap()
    wo_d = nc.dram_tensor("wo", [C, C], FP32, kind="ExternalInput").ap()
    w1_d = nc.dram_tensor("w1", [C, F], FP32, kind="ExternalInput").ap()
    w2_d = nc.dram_tensor("w2", [F, C], FP32, kind="ExternalInput").ap()
    id_d = nc.dram_tensor("iden", [128, 128], FP32, kind="ExternalInput").ap()
    qb_d = nc.dram_tensor("qb", [C], FP32, kind="ExternalInput").ap() if use_qb else None
    kb_d = nc.dram_tensor("kb", [C], FP32, kind="ExternalInput").ap() if use_kb else None
    vb_d = nc.dram_tensor("vb", [C], FP32, kind="ExternalInput").ap() if use_vb else None
    bo_d = nc.dram_tensor("bo_r", [C], FP32, kind="ExternalInput").ap() if use_bo else None
    b2_d = nc.dram_tensor("b2_r", [C], FP32, kind="ExternalInput").ap() if use_b2 else None
    b1_d = nc.dram_tensor("b1_r", [F], FP32, kind="ExternalInput").ap() if use_b1 else None
    out_d = nc.dram_tensor("out", [BC, T, C], FP32, kind="ExternalOutput").ap()

    from contextlib import ExitStack
    with tile.TileContext(nc) as tc, ExitStack() as ctx:
        wp = ctx.enter_context(tc.tile_pool(name="wpool", bufs=1))
        ap_ = ctx.enter_context(tc.tile_pool(name="apool", bufs=POOL_CFG["apool"]))
        sp = ctx.enter_context(tc.tile_pool(name="spool", bufs=POOL_CFG["spool"]))
        psA = ctx.enter_context(tc.tile_pool(name="psA", bufs=POOL_CFG["psA"], space="PSUM"))
        psT = ctx.enter_context(tc.tile_pool(name="psT", bufs=POOL_CFG["psT"], space="PSUM"))
        xp = ctx.enter_context(tc.tile_pool(name="xp", bufs=POOL_CFG.get("xpool", 3)))

        # ---- weights (fp16, cast during SWDGE DMA) ----
        wq_t = wp.tile([128, NC_, C], FP16, tag="wq")
        wk_t = wp.tile([128, NC_, C], FP16, tag="wk")
        wv_t = wp.tile([128, NC_, C], FP16, tag="wv")
        wo_t = wp.tile([128, NC_, C], FP16, tag="wo")
        w1_t = wp.tile([128, NC_, F], FP16, tag="w1")
        w2_t = wp.tile([128, NF, C], FP16, tag="w2")
        iden = wp.tile([128, 128], FP16, tag="iden")
        eps_t = wp.tile([128, 1], FP32, tag="eps")
        nc.vector.memset(eps_t, EPS)
        nc.gpsimd.dma_start(out=iden, in_=id_d)
        for wt, wd in ((wq_t, wq_d), (wk_t, wk_d), (wv_t, wv_d), (wo_t, wo_d)):
            nc.gpsimd.dma_start(out=wt, in_=wd.rearrange("(k p) n -> p k n", p=128))
        nc.gpsimd.dma_start(out=w1_t, in_=w1_d.rearrange("(k p) n -> p k n", p=128))
        nc.gpsimd.dma_start(out=w2_t, in_=w2_d.rearrange("(m p) n -> p m n", p=128))

        ones_row = None
        if use_vb or use_bo or use_b2:
            ones_row = wp.tile([1, 128], FP16, tag="ones_row")
            nc.vector.memset(ones_row, 1.0)
        qb_t = kb_t = None
        if use_qb:
            qb_t = wp.tile([128, NC_], FP32, tag="qb")
            nc.sync.dma_start(out=qb_t, in_=qb_d.rearrange("(m p) -> p m", p=128))
        if use_kb:
            kb_t = wp.tile([128, NC_], FP32, tag="kb")
            nc.sync.dma_start(out=kb_t, in_=kb_d.rearrange("(m p) -> p m", p=128))
        vb_t = bo_t = b2_t = b1_t = None
        if use_vb:
            vb_t = wp.tile([1, C], FP16, tag="vb")
            nc.gpsimd.dma_start(out=vb_t, in_=vb_d[None, :])
        if use_bo:
            bo_t = wp.tile([1, C], FP16, tag="bo")
            nc.gpsimd.dma_start(out=bo_t, in_=bo_d[None, :])
        if use_b2:
            b2_t = wp.tile([1, C], FP16, tag="b2")
            nc.gpsimd.dma_start(out=b2_t, in_=b2_d[None, :])
        if use_b1:
            b1_t = wp.tile([128, NF], FP32, tag="b1")
            nc.sync.dma_start(out=b1_t, in_=b1_d.rearrange("(m p) -> p m", p=128))

        def layer_norm(src, tag):
            mv = sp.tile([128, NT, 2], FP32, tag="mv")
            for i in range(NT):
                st = sp.tile([128, 6], FP32, tag="st")
                nc.vector.bn_stats(out=st, in_=src[:, i, :])
                nc.vector.bn_aggr(out=mv[:, i, :], in_=st)
            rstd = sp.tile([128, NT, 1], FP32, tag="rstd")
            if POOL_CFG.get("rstd_pow"):
                nc.gpsimd.tensor_scalar(
                    out=rstd, in0=mv[:, :, 1:2], scalar1=EPS, scalar2=-0.5,
                    op0=ALU.add, op1=ALU.pow)
            else:
                lnv = sp.tile([128, NT, 1], FP32, tag="lnv")
                nc.scalar.activation(out=lnv, in_=mv[:, :, 1:2], func=AF.Ln,
                                     bias=eps_t)
                nc.scalar.activation(out=rstd, in_=lnv, func=AF.Exp, scale=-0.5)
            dst = ap_.tile([128, NT, C], FP16, tag=tag)
            eng = nc.gpsimd if POOL_CFG.get("ln_apply_gpsimd") else nc.vector
            for i in range(NT):
                eng.tensor_scalar(
                    out=dst[:, i, :], in0=src[:, i, :],
                    scalar1=mv[:, i, 0:1], scalar2=rstd[:, i, :],
                    op0=ALU.subtract, op1=ALU.mult)
            return dst

        def stage_l(b):
            # ---- load x [128, 2, 384] (prefetched ahead of stage_a) ----
            if POOL_CFG.get("res16"):
                # fp16 residual stream: cast during DMA; halves DVE element
                # cost for LN stats/apply (2x 16-bit mode) + DMA bytes.
                x_t = xp.tile([128, NT, C], FP16, tag="x")
                eng = nc.gpsimd if POOL_CFG.get("res16_swdge") else nc.sync
                eng.dma_start(out=x_t,
                              in_=x_d[b].rearrange("(i p) c -> p i c", p=128))
            else:
                x_t = xp.tile([128, NT, C], FP32, tag="x")
                nc.sync.dma_start(out=x_t,
                                  in_=x_d[b].rearrange("(i p) c -> p i c", p=128))
            return x_t

        def stage_a(b, x_t):
            # ---- LN1 -> x1 fp16 token-major ----
            x1 = layer_norm(x_t, "x1")

            # ---- x1 -> feature-major x1f [128c, 3, 256t] ----
            x1f = ap_.tile([128, NC_, T], FP16, tag="x1f")
            x1f_evac = (nc.scalar.copy if "x1f" in POOL_CFG.get("tr_evac_scalar", "")
                        else nc.vector.tensor_copy)
            if POOL_CFG.get("tsplit"):
                # Per-t-half transpose banks: the copy for half 0 doesn't wait
                # on half 1's transposes (PSUM banks serialize at bank level).
                for i in range(NT):
                    p_tr = psT.tile([128, NC_, 128], FP16, tag="tr")
                    for j in range(NC_):
                        nc.tensor.transpose(p_tr[:, j, :],
                                            x1[:, i, 128 * j:128 * (j + 1)], iden)
                    x1f_evac(x1f[:, :, 128 * i:128 * (i + 1)], p_tr)
            else:
                p_tr = psT.tile([128, NC_, T], FP16, tag="tr")
                for i in range(NT):
                    for j in range(NC_):
                        nc.tensor.transpose(p_tr[:, j, 128 * i:128 * (i + 1)],
                                            x1[:, i, 128 * j:128 * (j + 1)], iden)
                x1f_evac(x1f, p_tr)

            # ---- q, k projections (feature-major out) ----
            if POOL_CFG.get("qk_merge") and qb_t is None and kb_t is None:
                # One [128, 2, T] PSUM bank per m-chunk holds q and k; one
                # ACT copy evacuates both. Halves psA pressure + evac count.
                qk_f = ap_.tile([128, 2, NC_, T], FP16, tag="qk_f")
                q_f = qk_f[:, 0]
                k_f = qk_f[:, 1]
                for m in range(NC_):
                    pq = psA.tile([128, 2, T], FP32, tag="a")
                    for qi, wt in ((0, wq_t), (1, wk_t)):
                        for kk in range(NC_):
                            nc.tensor.matmul(pq[:, qi, :],
                                             wt[:, kk, 128 * m:128 * (m + 1)],
                                             x1f[:, kk, :],
                                             start=(kk == 0), stop=(kk == NC_ - 1))
                    if POOL_CFG.get("qk_evac_dve"):
                        nc.vector.tensor_copy(qk_f[:, :, m, :], pq)
                    else:
                        nc.scalar.copy(out=qk_f[:, :, m, :], in_=pq)
            else:
                q_f = ap_.tile([128, NC_, T], FP16, tag="q_f")
                k_f = ap_.tile([128, NC_, T], FP16, tag="k_f")
                qk_halves = (((0, 128), (128, 128))
                             if POOL_CFG.get("tsplit") and not POOL_CFG.get("qk_nosplit")
                             else ((0, T),))
                for dst, wt, bias_t in ((q_f, wq_t, qb_t), (k_f, wk_t, kb_t)):
                    for m in range(NC_):
                        for lo, sz in qk_halves:
                            pq = psA.tile([128, T], FP32, tag="a")
                            for kk in range(NC_):
                                nc.tensor.matmul(pq[:, 0:sz],
                                                 wt[:, kk, 128 * m:128 * (m + 1)],
                                                 x1f[:, kk, lo:lo + sz],
                                                 start=(kk == 0), stop=(kk == NC_ - 1))
                            if bias_t is not None:
                                nc.scalar.activation(out=dst[:, m, lo:lo + sz],
                                                     in_=pq[:, 0:sz], func=AF.Identity,
                                                     bias=bias_t[:, m:m + 1])
                            elif POOL_CFG.get("qk_evac_dve"):
                                nc.vector.tensor_copy(dst[:, m, lo:lo + sz], pq[:, 0:sz])
                            else:
                                nc.scalar.copy(out=dst[:, m, lo:lo + sz], in_=pq[:, 0:sz])

            # ---- v projection (token-major, interleaved with ones col) ----
            v_t = ap_.tile([128, NT, H, HD + 1], FP16, tag="v_t")
            # ones column for the folded softmax denominator: one strided
            # gpsimd memset per batch, issued before the projections so it's
            # never on the critical path (and off the busy DVE queue).
            nc.gpsimd.memset(v_t[:, :, :, HD:HD + 1], 1.0)
            for i in range(NT):
                pv = psA.tile([128, C], FP32, tag="a")
                for kk in range(NC_):
                    nc.tensor.matmul(pv, x1f[:, kk, 128 * i:128 * (i + 1)],
                                     wv_t[:, kk, :], start=(kk == 0),
                                     stop=(kk == NC_ - 1 and vb_t is None))
                if vb_t is not None:
                    nc.tensor.matmul(pv, ones_row, vb_t, start=False, stop=True,
                                     skip_group_check=True)
                if POOL_CFG.get("v_evac_scalar"):
                    nc.scalar.copy(out=v_t[:, i, :, 0:HD],
                                   in_=pv.rearrange("p (h d) -> p h d", h=H))
                else:
                    nc.vector.tensor_copy(v_t[:, i, :, 0:HD],
                                          pv.rearrange("p (h d) -> p h d", h=H))

            # ---- scores (s-major), exp, causal mask ----
            if POOL_CFG.get("w384"):
                # Packed layout [128s, H, 384]: cols 0:256 are (si=0, t=0:256),
                # cols 256:384 are (si=1, t=128:256) — only the causally live
                # region is ever computed, exp'd, or masked.
                w_t = ap_.tile([128, H, 3 * 128], FP16, tag="w_t")
                for h in range(H):
                    j, r = h // 2, (h % 2) * 64
                    ps = psA.tile([128, 3 * 128], FP32, tag="a")
                    nc.tensor.matmul(ps[:, 0:T], k_f[r:r + 64, j, 0:128],
                                     q_f[r:r + 64, j, :], start=True, stop=True)
                    nc.tensor.matmul(ps[:, T:T + 128], k_f[r:r + 64, j, 128:T],
                                     q_f[r:r + 64, j, 128:T], start=True, stop=True)
                    nc.scalar.activation(out=w_t[:, h, :], in_=ps, func=AF.Exp,
                                         scale=SCALE)
                    for off in (0, 256):  # diagonal blocks: keep t-s >= 0
                        nc.gpsimd.affine_select(
                            out=w_t[:, h, off:off + 128],
                            in_=w_t[:, h, off:off + 128],
                            pattern=[[1, 128]], compare_op=ALU.is_ge,
                            fill=0.0, base=0, channel_multiplier=-1)
                return dict(x_t=x_t, w_t=w_t, v_t=v_t)
            w_t = ap_.tile([128, H, NT, T], FP16, tag="w_t")
            if POOL_CFG.get("head_pair"):
                for hp in range(H // 2):
                    ps = psA.tile([128, 2, NT, T], FP32, tag="a2")
                    for hh in range(2):
                        h = 2 * hp + hh
                        j, r = h // 2, (h % 2) * 64
                        nc.tensor.matmul(ps[:, hh, 0, :], k_f[r:r + 64, j, 0:128],
                                         q_f[r:r + 64, j, :], start=True, stop=True)
                        nc.tensor.matmul(ps[:, hh, 1, 128:T], k_f[r:r + 64, j, 128:T],
                                         q_f[r:r + 64, j, 128:T], start=True, stop=True)
                    nc.scalar.activation(out=w_t[:, 2 * hp:2 * hp + 2, :, :], in_=ps,
                                         func=AF.Exp, scale=SCALE)
                    nc.gpsimd.affine_select(
                        out=w_t[:, 2 * hp:2 * hp + 2, :, :],
                        in_=w_t[:, 2 * hp:2 * hp + 2, :, :],
                        pattern=[[0, 2], [-128, NT], [1, T]], compare_op=ALU.is_ge,
                        fill=0.0, base=0, channel_multiplier=-1)
            else:
                for h in range(H):
                    j, r = h // 2, (h % 2) * 64
                    ps = psA.tile([128, NT, T], FP32, tag="a")
                    nc.tensor.matmul(ps[:, 0, :], k_f[r:r + 64, j, 0:128],
                                     q_f[r:r + 64, j, :], start=True, stop=True)
                    nc.tensor.matmul(ps[:, 1, 128:T], k_f[r:r + 64, j, 128:T],
                                     q_f[r:r + 64, j, 128:T], start=True, stop=True)
                    nc.scalar.activation(out=w_t[:, h, :, :], in_=ps, func=AF.Exp,
                                         scale=SCALE)
                    nc.gpsimd.affine_select(
                        out=w_t[:, h, :, :], in_=w_t[:, h, :, :],
                        pattern=[[-128, NT], [1, T]], compare_op=ALU.is_ge,
                        fill=0.0, base=0, channel_multiplier=-1)

            return dict(x_t=x_t, w_t=w_t, v_t=v_t)

        def stage_m(b, hnd):
            x_t, w_t, v_t = hnd["x_t"], hnd["w_t"], hnd["v_t"]
            a_f = ap_.tile([128, NC_, T], FP16, tag="a_f")
            if POOL_CFG.get("fine_attn"):
                # Per (i, head-pair) processing: small PSUM tiles, reciprocal
                # and normalize per 128-col chunk, transpose immediately, and
                # per-i transpose banks so the a_f copy for t-chunk 0 doesn't
                # wait on t-chunk 1's transposes.
                soff = {(0, 0): 0, (1, 0): 128, (1, 1): 256}
                a_tok = ap_.tile([128, NT, C], FP16, tag="a_tok")
                for i in range(NT):
                    p_at = psT.tile([128, NC_, 128], FP16, tag="tr")
                    for j in range(NC_):
                        pa = psA.tile([128, 2, HD + 1], FP32, tag="a")
                        for hh in range(2):
                            h = 2 * j + hh
                            for si in range(i + 1):
                                o = soff[(i, si)]
                                nc.tensor.matmul(pa[:, hh, :],
                                                 w_t[:, h, o:o + 128],
                                                 v_t[:, si, h, :],
                                                 start=(si == 0), stop=(si == i))
                        r_t = sp.tile([128, 2, 1], FP32, tag="r_t")
                        nc.vector.reciprocal(r_t, pa[:, :, HD:HD + 1])
                        bcast = bass.AP(tensor=r_t.tensor, offset=r_t.offset,
                                        ap=[r_t.ap[0], [1, 2], [0, HD]])
                        nc.vector.tensor_tensor(
                            out=a_tok[:, i, 128 * j:128 * (j + 1)].rearrange(
                                "p (h d) -> p h d", h=2),
                            in0=pa[:, :, 0:HD], in1=bcast, op=ALU.mult)
                        nc.tensor.transpose(p_at[:, j, :],
                                            a_tok[:, i, 128 * j:128 * (j + 1)],
                                            iden)
                    if "a_f" in POOL_CFG.get("tr_evac_scalar", ""):
                        nc.scalar.copy(out=a_f[:, :, 128 * i:128 * (i + 1)],
                                       in_=p_at)
                    else:
                        nc.vector.tensor_copy(a_f[:, :, 128 * i:128 * (i + 1)],
                                              p_at)
            else:
                # ---- attention (token-major, Z in column 64) ----
                a_tok = ap_.tile([128, NT, C], FP16, tag="a_tok")
                a_f_evac = (nc.scalar.copy if "a_f" in POOL_CFG.get("tr_evac_scalar", "")
                            else nc.vector.tensor_copy)
                for i in range(NT):
                    pa = psA.tile([128, H, HD + 1], FP32, tag="a")
                    for h in range(H):
                        for si in range(i + 1):
                            w_src = (w_t[:, h, soff_legacy(i, si):soff_legacy(i, si) + 128]
                                     if POOL_CFG.get("w384") else
                                     w_t[:, h, si, 128 * i:128 * (i + 1)])
                            nc.tensor.matmul(pa[:, h, :], w_src,
                                             v_t[:, si, h, :],
                                             start=(si == 0), stop=(si == i))
                    if POOL_CFG.get("attn_evac_act") or POOL_CFG.get("attn_evac_act2"):
                        # Evacuate on ACT; normalize off-PSUM (gpsimd or
                        # all-fp16 DVE) to keep PSUM reads off the DVE path.
                        av = sp.tile([128, H, HD + 1], FP16, tag="av")
                        nc.scalar.copy(out=av, in_=pa)
                        r_t = sp.tile([128, H, 1], FP32, tag="r_t")
                        nc.vector.reciprocal(r_t, av[:, :, HD:HD + 1])
                        bcast = bass.AP(tensor=r_t.tensor, offset=r_t.offset,
                                        ap=[r_t.ap[0], [1, H], [0, HD]])
                        eng = (nc.vector if POOL_CFG.get("attn_evac_act2")
                               else nc.gpsimd)
                        eng.tensor_tensor(out=a_tok[:, i, :].rearrange(
                            "p (h d) -> p h d", h=H), in0=av[:, :, 0:HD],
                            in1=bcast, op=ALU.mult)
                    else:
                        r_t = sp.tile([128, H, 1], FP32, tag="r_t")
                        nc.vector.reciprocal(r_t, pa[:, :, HD:HD + 1])
                        bcast = bass.AP(tensor=r_t.tensor, offset=r_t.offset,
                                        ap=[r_t.ap[0], [1, H], [0, HD]])
                        nc.vector.tensor_tensor(out=a_tok[:, i, :].rearrange(
                            "p (h d) -> p h d", h=H), in0=pa[:, :, 0:HD], in1=bcast,
                            op=ALU.mult)
                    if POOL_CFG.get("tsplit") and POOL_CFG.get("attn_inline"):
                        # transpose+evacuate half i before issuing half i+1's
                        # matmuls: DVE order becomes recip0,tt0,copy0,recip1,…
                        p_at = psT.tile([128, NC_, 128], FP16, tag="tr")
                        for j in range(NC_):
                            nc.tensor.transpose(p_at[:, j, :],
                                                a_tok[:, i, 128 * j:128 * (j + 1)],
                                                iden)
                        a_f_evac(a_f[:, :, 128 * i:128 * (i + 1)], p_at)

                # ---- attn -> feature-major ----
                if POOL_CFG.get("tsplit") and POOL_CFG.get("attn_inline"):
                    pass  # done inline above
                elif POOL_CFG.get("tsplit"):
                    for i in range(NT):
                        p_at = psT.tile([128, NC_, 128], FP16, tag="tr")
                        for j in range(NC_):
                            nc.tensor.transpose(p_at[:, j, :],
                                                a_tok[:, i, 128 * j:128 * (j + 1)],
                                                iden)
                        a_f_evac(a_f[:, :, 128 * i:128 * (i + 1)], p_at)
                else:
                    p_at = psT.tile([128, NC_, T], FP16, tag="tr")
                    for i in range(NT):
                        for j in range(NC_):
                            nc.tensor.transpose(p_at[:, j, 128 * i:128 * (i + 1)],
                                                a_tok[:, i, 128 * j:128 * (j + 1)], iden)
                    a_f_evac(a_f, p_at)

            # ---- output projection + residual -> y ----
            y_t = ap_.tile([128, NT, C],
                           FP16 if POOL_CFG.get("res16") else FP32, tag="y_t")
            for i in range(NT):
                py = psA.tile([128, C], FP32, tag="a")
                for kk in range(NC_):
                    nc.tensor.matmul(py, a_f[:, kk, 128 * i:128 * (i + 1)],
                                     wo_t[:, kk, :], start=(kk == 0),
                                     stop=(kk == NC_ - 1 and bo_t is None))
                if bo_t is not None:
                    nc.tensor.matmul(py, ones_row, bo_t, start=False, stop=True,
                                     skip_group_check=True)
                nc.vector.tensor_tensor(out=y_t[:, i, :], in0=py, in1=x_t[:, i, :],
                                        op=ALU.add)

            # ---- LN2 -> x2 fp16 ----
            x2 = layer_norm(y_t, "x2")

            # ---- x2 -> feature-major ----
            x2f = ap_.tile([128, NC_, T], FP16, tag="x2f")
            x2f_evac = (nc.scalar.copy if "x2f" in POOL_CFG.get("tr_evac_scalar", "")
                        else nc.vector.tensor_copy)
            if POOL_CFG.get("tsplit"):
                for i in range(NT):
                    p_x2 = psT.tile([128, NC_, 128], FP16, tag="tr")
                    for j in range(NC_):
                        nc.tensor.transpose(p_x2[:, j, :],
                                            x2[:, i, 128 * j:128 * (j + 1)], iden)
                    x2f_evac(x2f[:, :, 128 * i:128 * (i + 1)], p_x2)
            else:
                p_x2 = psT.tile([128, NC_, T], FP16, tag="tr")
                for i in range(NT):
                    for j in range(NC_):
                        nc.tensor.transpose(p_x2[:, j, 128 * i:128 * (i + 1)],
                                            x2[:, i, 128 * j:128 * (j + 1)], iden)
                x2f_evac(x2f, p_x2)
            return dict(x2f=x2f, y_t=y_t)

        def stage_f(b, hnd):
            x2f, y_t = hnd["x2f"], hnd["y_t"]
            # ---- FFN1 + ReLU -> h_t fp16 (feature-major) ----
            h_t = ap_.tile([128, NF, T], FP16, tag="h_t")
            f1_halves = ((0, 128), (128, 128)) if POOL_CFG.get("ffn1_split") else ((0, T),)
            for mp in range(NF // 2):
                ph = psA.tile([128, 2, T], FP32, tag="a")
                for sub in range(2):
                    m = 2 * mp + sub
                    for lo, sz in f1_halves:
                        for kk in range(NC_):
                            nc.tensor.matmul(ph[:, sub, lo:lo + sz],
                                             w1_t[:, kk, 128 * m:128 * (m + 1)],
                                             x2f[:, kk, lo:lo + sz],
                                             start=(kk == 0), stop=(kk == NC_ - 1))
                if b1_t is not None:
                    for sub in range(2):
                        m = 2 * mp + sub
                        nc.vector.tensor_scalar(
                            out=h_t[:, m, :], in0=ph[:, sub, :],
                            scalar1=b1_t[:, m:m + 1], scalar2=0.0,
                            op0=ALU.add, op1=ALU.max)
                elif mp < POOL_CFG.get("relu_dve", 3):
                    nc.vector.tensor_scalar(
                        out=h_t[:, 2 * mp:2 * mp + 2, :], in0=ph,
                        scalar1=0.0, scalar2=None, op0=ALU.max)
                else:
                    nc.scalar.activation(out=h_t[:, 2 * mp:2 * mp + 2, :], in_=ph,
                                         func=AF.Relu)

            # ---- FFN2 + residual -> out ----
            res16 = POOL_CFG.get("res16")
            o_t = ap_.tile([128, NT, C], FP16 if res16 else FP32, tag="o_t")
            for i in range(NT):
                po = psA.tile([128, C], FP32, tag="a")
                for m in range(NF):
                    nc.tensor.matmul(po, h_t[:, m, 128 * i:128 * (i + 1)],
                                     w2_t[:, m, :], start=(m == 0),
                                     stop=(m == NF - 1 and b2_t is None))
                if b2_t is not None:
                    nc.tensor.matmul(po, ones_row, b2_t, start=False, stop=True,
                                     skip_group_check=True)
                nc.vector.tensor_tensor(out=o_t[:, i, :], in0=po, in1=y_t[:, i, :],
                                        op=ALU.add)

            out_ap = out_d[b].rearrange("(i p) c -> p i c", p=128)
            if res16 and POOL_CFG.get("res16_swdge"):
                nc.gpsimd.dma_start(out=out_ap, in_=o_t)  # cast fp16 -> fp32
            else:
                nc.sync.dma_start(out=out_ap, in_=o_t)

        # ===== FLAG3: sub-stage software pipeline ============================
        def fl3_a1(S):
            S["x1"] = layer_norm(S["x_t"], "x1")

        def fl3_a2(S, filler=None):
            x1 = S["x1"]
            x1f = ap_.tile([128, NC_, T], FP16, tag="x1f")
            if POOL_CFG.get("dma_tr_x"):
                # SBUF->SBUF transpose on the DMA xbar: no PE matmul, no PSUM
                # bank, no DVE evacuation copy.
                for i in range(NT):
                    for j in range(NC_):
                        nc.sync.dma_start_transpose(
                            x1f[:, j, 128 * i:128 * (i + 1)],
                            x1[:, i, 128 * j:128 * (j + 1)])
            else:
                for i in range(NT):
                    p_tr = psT.tile([128, NC_, 128], FP16, tag="tr")
                    for j in range(NC_):
                        nc.tensor.transpose(p_tr[:, j, :],
                                            x1[:, i, 128 * j:128 * (j + 1)], iden)
                    nc.vector.tensor_copy(x1f[:, :, 128 * i:128 * (i + 1)], p_tr)
                    if filler is not None and i == 0:
                        filler()
            S["x1f"] = x1f

        def fl3_a12(S):
            fl3_a1(S)
            fl3_a2(S)

        def fl3_a3(S):
            # q/k (merged psum) + v projections
            x1f = S["x1f"]
            qk_f = ap_.tile([128, 2, NC_, T], FP16, tag="qk_f")
            for m in range(NC_):
                pq = psA.tile([128, 2, T], FP32, tag="a")
                for qi, wt in ((0, wq_t), (1, wk_t)):
                    for kk in range(NC_):
                        nc.tensor.matmul(pq[:, qi, :],
                                         wt[:, kk, 128 * m:128 * (m + 1)],
                                         x1f[:, kk, :],
                                         start=(kk == 0), stop=(kk == NC_ - 1))
                if POOL_CFG.get("qk_evac_dve"):
                    nc.vector.tensor_copy(qk_f[:, :, m, :], pq)
                else:
                    nc.scalar.copy(out=qk_f[:, :, m, :], in_=pq)
            v_t = ap_.tile([128, NT, H, HD + 1], FP16, tag="v_t")
            nc.gpsimd.memset(v_t[:, :, :, HD:HD + 1], 1.0)
            for i in range(NT):
                pv = psA.tile([128, C], FP32, tag="a")
                for kk in range(NC_):
                    nc.tensor.matmul(pv, x1f[:, kk, 128 * i:128 * (i + 1)],
                                     wv_t[:, kk, :], start=(kk == 0),
                                     stop=(kk == NC_ - 1))
                if POOL_CFG.get("v_evac_scalar"):
                    nc.scalar.copy(out=v_t[:, i, :, 0:HD],
                                   in_=pv.rearrange("p (h d) -> p h d", h=H))
                else:
                    nc.vector.tensor_copy(v_t[:, i, :, 0:HD],
                                          pv.rearrange("p (h d) -> p h d", h=H))
            S["qk_f"], S["v_t"] = qk_f, v_t

        def fl3_scores_head(S, h, part=None):
            qk_f = S["qk_f"]
            q_f, k_f = qk_f[:, 0], qk_f[:, 1]
            w_t = S["w_t"]
            j, r = h // 2, (h % 2) * 64
            if part is None or part == 0:
                ps = psA.tile([128, 3 * 128], FP32, tag="a")
                S.setdefault("ps_h", {})[h] = ps
                nc.tensor.matmul(ps[:, 0:T], k_f[r:r + 64, j, 0:128],
                                 q_f[r:r + 64, j, :], start=True, stop=True)
                nc.tensor.matmul(ps[:, T:T + 128], k_f[r:r + 64, j, 128:T],
                                 q_f[r:r + 64, j, 128:T], start=True, stop=True)
            ps = S["ps_h"][h] if part is not None else ps
            if part is None:
                nc.scalar.activation(out=w_t[:, h, :], in_=ps, func=AF.Exp,
                                     scale=SCALE)
                offs = (0, 256)
            elif part == 0:
                # i=0-critical block only: attn(i=0) reads w_t[:, h, 0:128]
                nc.scalar.activation(out=w_t[:, h, 0:128], in_=ps[:, 0:128],
                                     func=AF.Exp, scale=SCALE)
                offs = (0,)
            else:
                nc.scalar.activation(out=w_t[:, h, 128:384], in_=ps[:, 128:384],
                                     func=AF.Exp, scale=SCALE)
                offs = (256,)
            for off in offs:
                nc.gpsimd.affine_select(
                    out=w_t[:, h, off:off + 128], in_=w_t[:, h, off:off + 128],
                    pattern=[[1, 128]], compare_op=ALU.is_ge,
                    fill=0.0, base=0, channel_multiplier=-1)

        def fl3_a4(S):
            # scores (packed w384), exp, diagonal-only causal mask
            w_t = ap_.tile([128, H, 3 * 128], FP16, tag="w_t")
            S["w_t"] = w_t
            if POOL_CFG.get("score_rowtile"):
                # K=64 per head: pack head pairs into disjoint PE row groups
                # (rows 0-63 / 64-127 via tile_position) so the two heads'
                # matmuls run concurrently in the array — halves score time.
                qk_f = S["qk_f"]
                q_f, k_f = qk_f[:, 0], qk_f[:, 1]
                for j in range(NC_):
                    ps_lo = psA.tile([128, 3 * 128], FP32, tag="a")
                    ps_hi = psA.tile([128, 3 * 128], FP32, tag="a")
                    pss = (ps_lo, ps_hi)
                    # (out_lo, out_sz, k_lo, q_lo): chunkA = s 0:128 x t 0:256;
                    # chunkB = s 128:256 x t 128:256 (packed at cols 256:384)
                    for out_lo, out_sz, k_lo, q_lo in ((0, T, 0, 0),
                                                       (T, 128, 128, 128)):
                        for r, ps in ((0, pss[0]), (64, pss[1])):
                            nc.tensor.matmul(
                                ps[:, out_lo:out_lo + out_sz],
                                k_f[r:r + 64, j, k_lo:k_lo + 128],
                                q_f[r:r + 64, j, q_lo:q_lo + out_sz],
                                start=True, stop=True, tile_position=(r, 0))
                    for hh, ps in enumerate(pss):
                        h = 2 * j + hh
                        nc.scalar.activation(out=w_t[:, h, :], in_=ps,
                                             func=AF.Exp, scale=SCALE)
                        for off in (0, 256):
                            nc.gpsimd.affine_select(
                                out=w_t[:, h, off:off + 128],
                                in_=w_t[:, h, off:off + 128],
                                pattern=[[1, 128]], compare_op=ALU.is_ge,
                                fill=0.0, base=0, channel_multiplier=-1)
                return
            if POOL_CFG.get("exp_split"):
                for h in range(H):
                    fl3_scores_head(S, h, part=0)
                for h in range(H):
                    fl3_scores_head(S, h, part=1)
                S.pop("ps_h")
            else:
                for h in range(H):
                    fl3_scores_head(S, h)

        def fl3_a34(S):
            # fused: per m-chunk, project q/k, evacuate, then immediately run
            # the two heads living in that chunk through scores+exp+mask —
            # head 0's exp no longer queues behind all three evacuations.
            x1f = S["x1f"]
            qk_f = ap_.tile([128, 2, NC_, T], FP16, tag="qk_f")
            S["qk_f"] = qk_f
            w_t = ap_.tile([128, H, 3 * 128], FP16, tag="w_t")
            S["w_t"] = w_t
            for m in range(NC_):
                pq = psA.tile([128, 2, T], FP32, tag="a")
                for qi, wt in ((0, wq_t), (1, wk_t)):
                    for kk in range(NC_):
                        nc.tensor.matmul(pq[:, qi, :],
                                         wt[:, kk, 128 * m:128 * (m + 1)],
                                         x1f[:, kk, :],
                                         start=(kk == 0), stop=(kk == NC_ - 1))
                nc.scalar.copy(out=qk_f[:, :, m, :], in_=pq)
                fl3_scores_head(S, 2 * m)
                fl3_scores_head(S, 2 * m + 1)
            v_t = ap_.tile([128, NT, H, HD + 1], FP16, tag="v_t")
            nc.gpsimd.memset(v_t[:, :, :, HD:HD + 1], 1.0)
            for i in range(NT):
                pv = psA.tile([128, C], FP32, tag="a")
                for kk in range(NC_):
                    nc.tensor.matmul(pv, x1f[:, kk, 128 * i:128 * (i + 1)],
                                     wv_t[:, kk, :], start=(kk == 0),
                                     stop=(kk == NC_ - 1))
                nc.vector.tensor_copy(v_t[:, i, :, 0:HD],
                                      pv.rearrange("p (h d) -> p h d", h=H))
            S["v_t"] = v_t

        def fl3_a34c(S):
            # pipelined qk/v/scores: evacuations overlap later projections and
            # each head-pair's exp issues as early as the data allows, so the
            # ACT queue drains w_t before the next iteration's attn matmuls.
            x1f = S["x1f"]
            qk_f = ap_.tile([128, 2, NC_, T], FP16, tag="qk_f")
            S["qk_f"] = qk_f
            w_t = ap_.tile([128, H, 3 * 128], FP16, tag="w_t")
            S["w_t"] = w_t
            q_f, k_f = qk_f[:, 0], qk_f[:, 1]
            v_t = ap_.tile([128, NT, H, HD + 1], FP16, tag="v_t")
            nc.gpsimd.memset(v_t[:, :, :, HD:HD + 1], 1.0)
            S["v_t"] = v_t

            def qk_mm(m):
                pq = psA.tile([128, 2, T], FP32, tag="a")
                for qi, wt in ((0, wq_t), (1, wk_t)):
                    for kk in range(NC_):
                        nc.tensor.matmul(pq[:, qi, :],
                                         wt[:, kk, 128 * m:128 * (m + 1)],
                                         x1f[:, kk, :],
                                         start=(kk == 0), stop=(kk == NC_ - 1))
                nc.scalar.copy(out=qk_f[:, :, m, :], in_=pq)

            def v_mm(i):
                pv = psA.tile([128, C], FP32, tag="a")
                for kk in range(NC_):
                    nc.tensor.matmul(pv, x1f[:, kk, 128 * i:128 * (i + 1)],
                                     wv_t[:, kk, :], start=(kk == 0),
                                     stop=(kk == NC_ - 1))
                nc.vector.tensor_copy(v_t[:, i, :, 0:HD],
                                      pv.rearrange("p (h d) -> p h d", h=H))

            def sc(j):
                ps_lo = psA.tile([128, 3 * 128], FP32, tag="a")
                ps_hi = psA.tile([128, 3 * 128], FP32, tag="a")
                for out_lo, out_sz, k_lo, q_lo in ((0, T, 0, 0),
                                                   (T, 128, 128, 128)):
                    for r, ps in ((0, ps_lo), (64, ps_hi)):
                        nc.tensor.matmul(
                            ps[:, out_lo:out_lo + out_sz],
                            k_f[r:r + 64, j, k_lo:k_lo + 128],
                            q_f[r:r + 64, j, q_lo:q_lo + out_sz],
                            start=True, stop=True, tile_position=(r, 0))
                for hh, ps in ((0, ps_lo), (1, ps_hi)):
                    h = 2 * j + hh
                    nc.scalar.activation(out=w_t[:, h, :], in_=ps,
                                         func=AF.Exp, scale=SCALE)
                    for off in (0, 256):
                        nc.gpsimd.affine_select(
                            out=w_t[:, h, off:off + 128],
                            in_=w_t[:, h, off:off + 128],
                            pattern=[[1, 128]], compare_op=ALU.is_ge,
                            fill=0.0, base=0, channel_multiplier=-1)

            qk_mm(0); qk_mm(1); v_mm(0); sc(0); qk_mm(2); v_mm(1); sc(1); sc(2)

        def fl3_mattn_mm_i(S, i, filler=None):
            w_t, v_t = S["w_t"], S["v_t"]
            soff = {(0, 0): 0, (1, 0): 128, (1, 1): 256}
            if i == 0:
                a_tok = ap_.tile([128, NT, C], FP16, tag="a_tok")
                S["a_tok"] = a_tok
            a_tok = S["a_tok"]
            pa = psA.tile([128, H, HD + 1], FP32, tag="a")
            for h in range(H):
                for si in range(i + 1):
                    o = soff[(i, si)]
                    nc.tensor.matmul(pa[:, h, :], w_t[:, h, o:o + 128],
                                     v_t[:, si, h, :],
                                     start=(si == 0), stop=(si == i))
                if filler is not None and (i == 1 or h % 2 == 1) and h < H - 1:
                    filler(h // 2)  # FFN filler between attn weight loads
            r_t = sp.tile([128, H, 1], FP32, tag="r_t")
            nc.vector.reciprocal(r_t, pa[:, :, HD:HD + 1])
            bcast = bass.AP(tensor=r_t.tensor, offset=r_t.offset,
                            ap=[r_t.ap[0], [1, H], [0, HD]])
            nc.vector.tensor_tensor(out=a_tok[:, i, :].rearrange(
                "p (h d) -> p h d", h=H), in0=pa[:, :, 0:HD], in1=bcast,
                op=ALU.mult)

        def fl3_mattn_mm(S):
            for i in range(NT):
                fl3_mattn_mm_i(S, i)

        def fl3_mattn_tr(S, filler=None):
            a_tok = S["a_tok"]
            a_f = ap_.tile([128, NC_, T], FP16, tag="a_f")
            if POOL_CFG.get("dma_tr_a"):
                for i in range(NT):
                    for j in range(NC_):
                        nc.sync.dma_start_transpose(
                            a_f[:, j, 128 * i:128 * (i + 1)],
                            a_tok[:, i, 128 * j:128 * (j + 1)])
            else:
                for i in range(NT):
                    p_at = psT.tile([128, NC_, 128], FP16, tag="tr")
                    for j in range(NC_):
                        nc.tensor.transpose(p_at[:, j, :],
                                            a_tok[:, i, 128 * j:128 * (j + 1)], iden)
                    nc.vector.tensor_copy(a_f[:, :, 128 * i:128 * (i + 1)], p_at)
                    if filler is not None and i == 0:
                        filler()
            S["a_f"] = a_f

        def fl3_mattn(S):
            fl3_mattn_mm(S)
            fl3_mattn_tr(S)

        def fl3_m3(S):
            # Wo projection + residual + LN2
            a_f, x_t = S["a_f"], S["x_t"]
            y_t = ap_.tile([128, NT, C], FP32, tag="y_t")
            for i in range(NT):
                py = psA.tile([128, C], FP32, tag="a")
                for kk in range(NC_):
                    nc.tensor.matmul(py, a_f[:, kk, 128 * i:128 * (i + 1)],
                                     wo_t[:, kk, :], start=(kk == 0),
                                     stop=(kk == NC_ - 1))
                nc.vector.tensor_tensor(out=y_t[:, i, :], in0=py,
                                        in1=x_t[:, i, :], op=ALU.add)
            S["y_t"] = y_t
            S["x2"] = layer_norm(y_t, "x2")

        def fl3_m4(S, filler=None):
            x2 = S["x2"]
            x2f = ap_.tile([128, NC_, T], FP16, tag="x2f")
            if POOL_CFG.get("dma_tr_x"):
                for i in range(NT):
                    for j in range(NC_):
                        nc.sync.dma_start_transpose(
                            x2f[:, j, 128 * i:128 * (i + 1)],
                            x2[:, i, 128 * j:128 * (j + 1)])
            else:
                for i in range(NT):
                    p_x2 = psT.tile([128, NC_, 128], FP16, tag="tr")
                    for j in range(NC_):
                        nc.tensor.transpose(p_x2[:, j, :],
                                            x2[:, i, 128 * j:128 * (j + 1)], iden)
                    nc.vector.tensor_copy(x2f[:, :, 128 * i:128 * (i + 1)], p_x2)
                    if filler is not None and i == 0:
                        filler()
            S["x2f"] = x2f

        def fl3_f1(S, mp_lo=0, mp_hi=NF // 2):
            # FFN1 + ReLU (ACT so LN1(k+1) can start on DVE underneath)
            x2f = S["x2f"]
            if mp_lo == 0:
                h_t = ap_.tile([128, NF, T], FP16, tag="h_t")
                S["h_t"] = h_t
            h_t = S["h_t"]
            for mp in range(mp_lo, mp_hi):
                ph = psA.tile([128, 2, T], FP32, tag="a")
                for sub in range(2):
                    m = 2 * mp + sub
                    for kk in range(NC_):
                        nc.tensor.matmul(ph[:, sub, :],
                                         w1_t[:, kk, 128 * m:128 * (m + 1)],
                                         x2f[:, kk, :],
                                         start=(kk == 0), stop=(kk == NC_ - 1))
                if mp < POOL_CFG.get("fl3_relu_dve", 0):
                    nc.vector.tensor_scalar(
                        out=h_t[:, 2 * mp:2 * mp + 2, :], in0=ph,
                        scalar1=0.0, scalar2=None, op0=ALU.max)
                else:
                    nc.scalar.activation(out=h_t[:, 2 * mp:2 * mp + 2, :],
                                         in_=ph, func=AF.Relu)

        def fl3_f1m(S, m):
            # single-m FFN1 unit (3 matmuls N=256 + one ReLU evac): the
            # filler currency for hiding small-MM weight loads elsewhere.
            x2f = S["x2f"]
            if m == 0:
                h_t = ap_.tile([128, NF, T], FP16, tag="h_t")
                S["h_t"] = h_t
            h_t = S["h_t"]
            ph1 = psA.tile([128, T], FP32, tag="a")
            for kk in range(NC_):
                nc.tensor.matmul(ph1, w1_t[:, kk, 128 * m:128 * (m + 1)],
                                 x2f[:, kk, :], start=(kk == 0),
                                 stop=(kk == NC_ - 1))
            if (m // 2) < POOL_CFG.get("fl3_relu_dve", 0):
                nc.vector.tensor_scalar(out=h_t[:, m, :], in0=ph1,
                                        scalar1=0.0, scalar2=None, op0=ALU.max)
            else:
                nc.scalar.activation(out=h_t[:, m, :], in_=ph1, func=AF.Relu)

        def fl3_f2(S):
            h_t, y_t = S["h_t"], S["y_t"]
            o_t = ap_.tile([128, NT, C], FP32, tag="o_t")
            for i in range(NT):
                po = psA.tile([128, C], FP32, tag="a")
                for m in range(NF):
                    nc.tensor.matmul(po, h_t[:, m, 128 * i:128 * (i + 1)],
                                     w2_t[:, m, :], start=(m == 0),
                                     stop=(m == NF - 1))
                nc.vector.tensor_tensor(out=o_t[:, i, :], in0=po,
                                        in1=y_t[:, i, :], op=ALU.add)
            nc.sync.dma_start(out=out_d[S["b"]].rearrange("(i p) c -> p i c", p=128),
                              in_=o_t)

        order = [bb for _ in range(repeat) for bb in range(BC)]
        n = len(order)
        la = POOL_CFG.get("lookahead", 1)
        xla = POOL_CFG.get("xla", 2)
        seq = POOL_CFG.get("order", "AMF")
        xh = {}

        def ensure_x(upto):
            for q in range(min(upto + 1, n)):
                if q not in xh:
                    xh[q] = stage_l(order[q])

        def run_a(j):
            ensure_x(j + xla)
            return stage_a(order[j], xh.pop(j))

        if seq in ("FLAG3", "FLAG4", "FLAG5", "FLAG6"):
            SS = {}

            def getS(j):
                if j not in SS:
                    ensure_x(j + xla)
                    SS[j] = {"b": order[j], "x_t": xh.pop(j)}
                return SS[j]

            if seq == "FLAG6":
                # pair-stepped FLAG5: two batches per pipeline slot so each
                # cross-engine sync latency is amortized over 2x PE filler.
                assert n % 2 == 0
                HALF = NF // 4
                for j in (0, 1):
                    fl3_a12(getS(j)); fl3_a3(getS(j)); fl3_a4(getS(j))
                for k in range(0, n, 2):
                    ks = (k, k + 1)
                    nxt = [j for j in (k + 2, k + 3) if j < n]
                    prv = [j for j in (k - 2, k - 1) if j >= 0]
                    for j in nxt:
                        fl3_a1(getS(j))
                    for j in prv:
                        fl3_f1(SS[j], 0, HALF)
                    for j in ks:
                        fl3_mattn_mm(SS[j])
                    for j in prv:
                        fl3_f1(SS[j], HALF, NF // 2)
                    for j in ks:
                        fl3_mattn_tr(SS[j])
                    for j in nxt:
                        fl3_a2(SS[j])
                    for j in ks:
                        fl3_m3(SS[j])
                    for j in prv:
                        fl3_f2(SS.pop(j))
                    for j in ks:
                        fl3_m4(SS[j])
                    for j in nxt:
                        fl3_a3(SS[j])
                    for j in nxt:
                        fl3_a4(SS[j])
                for j in (n - 2, n - 1):
                    fl3_f1(SS[j])
                for j in (n - 2, n - 1):
                    fl3_f2(SS.pop(j))
                seq = "DONE"

            # prologue: batch 0 fully through stage-a
            if seq != "DONE":
                fl3_a12(getS(0)); fl3_a3(getS(0)); fl3_a4(getS(0))
            if seq == "FLAG5":
                HALF = NF // 4  # FFN1 pair-split point
                a2_early = POOL_CFG.get("fl5_a2_early")
                m4_early = POOL_CFG.get("fl5_m4_early")
                three = POOL_CFG.get("fl5_f1_thirds")
                ln_deep = POOL_CFG.get("fl5_ln_deep")
                if ln_deep and n > 1:
                    fl3_a1(getS(1))
                for k in range(n):
                    if k + 1 < n:
                        fl3_a1(getS(k + 2 if ln_deep else k + 1)) \
                            if (not ln_deep or k + 2 < n) else None
                    s1, s2, s3 = POOL_CFG.get("fl5_splits", (2, 3, 4))
                    if POOL_CFG.get("fl5_head_ilv2"):
                        # head_ilv variant: same coarse mp-granular fillers,
                        # but one unit threaded between each transpose pair
                        # (mattn_tr / a2) to hide those LDWEIGHTS too.
                        prev = SS[k - 1] if k - 1 >= 0 else None
                        mp_st = {"mp": 0}

                        def fmp(_h=None):
                            if prev is not None and mp_st["mp"] < NF // 2:
                                fl3_f1(prev, mp_st["mp"], mp_st["mp"] + 1)
                                mp_st["mp"] += 1
                        fl3_mattn_mm_i(SS[k], 0, filler=fmp)
                        fmp()
                        fl3_mattn_mm_i(SS[k], 1)
                        fmp()
                        fl3_mattn_tr(SS[k], filler=fmp)
                        if k + 1 < n:
                            fl3_a2(SS[k + 1], filler=fmp)
                        fl3_m3(SS[k])
                        fl3_m4(SS[k])
                        while prev is not None and mp_st["mp"] < NF // 2:
                            fmp()
                        if k + 1 < n:
                            fl3_a3(SS[k + 1])
                            fl3_a4(SS[k + 1])
                        if prev is not None:
                            fl3_f2(SS.pop(k - 1))
                        continue
                    if POOL_CFG.get("f1m_ilv"):
                        # m-granular FFN1 fillers woven between every small-N
                        # PE instruction group (attn heads, transpose halves)
                        # so their LDWEIGHTS stream behind real matmul work.
                        prev = SS[k - 1] if k - 1 >= 0 else None
                        mst = {"m": 0}

                        def fm(_h=None, n_units=1):
                            if prev is None:
                                return
                            while n_units > 0 and mst["m"] < NF:
                                fl3_f1m(prev, mst["m"])
                                mst["m"] += 1
                                n_units -= 1
                        fl3_mattn_mm_i(SS[k], 0, filler=fm)
                        fm()
                        fl3_mattn_mm_i(SS[k], 1, filler=fm)
                        fm()
                        fl3_mattn_tr(SS[k], filler=fm)
                        if k + 1 < n:
                            fl3_a2(SS[k + 1], filler=fm)
                        fl3_m3(SS[k])
                        fl3_m4(SS[k], filler=fm)
                        fm(n_units=NF)  # drain whatever's left
                        if k + 1 < n:
                            fl3_a3(SS[k + 1])
                            fl3_a4(SS[k + 1])
                        if prev is not None:
                            fl3_f2(SS.pop(k - 1))
                        continue
                    if (POOL_CFG.get("fl5_hilv_i1") and k - 1 >= 0
                            and POOL_CFG.get("fl5_attn_split")):
                        # head_ilv + fillers between i=1 attn head-pairs too
                        # (use fl5_splits=(2,3,6) so the post-tr f1 is empty)
                        prev = SS[k - 1]
                        fl3_mattn_mm_i(
                            SS[k], 0,
                            filler=lambda jj, p=prev: fl3_f1(p, jj, jj + 1))
                        fl3_f1(prev, 2, 3)
                        i1_cnt = {"i": 0}

                        def i1f(jj, p=prev):
                            i1_cnt["i"] += 1
                            if i1_cnt["i"] in (2, 4):  # after heads 1 and 3
                                m = 2 + i1_cnt["i"] // 2  # mp 3, then 4
                                fl3_f1(p, m, m + 1)
                        fl3_mattn_mm_i(SS[k], 1, filler=i1f)
                        fl3_f1(prev, 5, 6)
                    elif (POOL_CFG.get("fl5_head_ilv") and k - 1 >= 0
                            and POOL_CFG.get("fl5_attn_split")):
                        prev = SS[k - 1]
                        fl3_mattn_mm_i(
                            SS[k], 0,
                            filler=lambda jj, p=prev: fl3_f1(p, jj, jj + 1))
                        fl3_f1(prev, 2, s2)
                        fl3_mattn_mm_i(SS[k], 1)
                        fl3_f1(prev, s2, s3)
                    elif POOL_CFG.get("fl5_attn_split") and k - 1 >= 0:
                        fl3_f1(SS[k - 1], 0, s1)
                        fl3_mattn_mm_i(SS[k], 0)
                        fl3_f1(SS[k - 1], s1, s2)
                        fl3_mattn_mm_i(SS[k], 1)
                        fl3_f1(SS[k - 1], s2, s3)
                    else:
                        if k - 1 >= 0:
                            fl3_f1(SS[k - 1], 0, 2 if three else HALF)
                        fl3_mattn_mm(SS[k])
                        if k - 1 >= 0:
                            fl3_f1(SS[k - 1], 2 if three else HALF,
                                   4 if three else NF // 2)
                    if a2_early and k + 1 < n:
                        fl3_a2(SS[k + 1])
                    fl3_mattn_tr(SS[k])
                    if k - 1 >= 0:
                        if POOL_CFG.get("fl5_attn_split"):
                            fl3_f1(SS[k - 1], s3, NF // 2)
                        elif three:
                            fl3_f1(SS[k - 1], 4, NF // 2)
                    if not a2_early and k + 1 < n:
                        fl3_a2(SS[k + 1])
                    fl3_m3(SS[k])
                    f2_pos = POOL_CFG.get("fl5_f2_pos", 0)
                    if k - 1 >= 0 and f2_pos == 0:
                        fl3_f2(SS.pop(k - 1))
                    if POOL_CFG.get("fl5_fuse34"):
                        fl3_m4(SS[k])
                        if k + 1 < n:
                            fl3_a34(SS[k + 1])
                    elif m4_early:
                        fl3_m4(SS[k])
                        if k + 1 < n:
                            if POOL_CFG.get("a34c"):
                                fl3_a34c(SS[k + 1])
                            else:
                                fl3_a3(SS[k + 1])
                        if k - 1 >= 0 and f2_pos == 1:
                            fl3_f2(SS.pop(k - 1))
                        if k + 1 < n and not POOL_CFG.get("a34c"):
                            fl3_a4(SS[k + 1])
                        if k - 1 >= 0 and f2_pos == 2:
                            fl3_f2(SS.pop(k - 1))
                    else:
                        if k + 1 < n:
                            fl3_a3(SS[k + 1])
                        fl3_m4(SS[k])
                        if k + 1 < n:
                            fl3_a4(SS[k + 1])
                fl3_f1(SS[n - 1]); fl3_f2(SS.pop(n - 1))
            elif seq == "FLAG4":
                for k in range(n):
                    if k + 1 < n:
                        fl3_a12(getS(k + 1))
                    if k - 1 >= 0:
                        fl3_f1(SS[k - 1])
                    fl3_mattn(SS[k])
                    fl3_m3(SS[k])
                    if k - 1 >= 0:
                        fl3_f2(SS.pop(k - 1))
                    if k + 1 < n:
                        fl3_a3(SS[k + 1])
                    fl3_m4(SS[k])
                    if k + 1 < n:
                        fl3_a4(SS[k + 1])
                fl3_f1(SS[n - 1]); fl3_f2(SS.pop(n - 1))
            elif seq == "FLAG3":
                for k in range(n):
                    fl3_mattn(SS[k])
                    fl3_m3(SS[k])
                    if k - 1 >= 0:
                        fl3_f1(SS[k - 1])
                    fl3_m4(SS[k])
                    if k + 1 < n:
                        fl3_a12(getS(k + 1))
                    if k - 1 >= 0:
                        fl3_f2(SS.pop(k - 1))
                    if k + 1 < n:
                        fl3_a3(SS[k + 1])
                        fl3_a4(SS[k + 1])
                fl3_f1(SS[n - 1]); fl3_f2(SS.pop(n - 1))
            seq = "DONE"

        ah = {}
        for j in range(min(la + 1, n) if seq != "DONE" else 0):
            ah[j] = run_a(j)
        if seq == "FLAG1":
            mh = {}
            for idx in range(n):
                nxt = idx + la + 1
                if nxt < n: ah[nxt] = run_a(nxt)
                mh[idx] = stage_m(order[idx], ah.pop(idx))
                if idx - 1 in mh:
                    stage_f(order[idx - 1], mh.pop(idx - 1))
            stage_f(order[n - 1], mh.pop(n - 1))
        for idx in (range(n) if seq not in ("FLAG1", "DONE") else []):
            nxt = idx + la + 1
            if seq == "AMF":
                if nxt < n: ah[nxt] = run_a(nxt)
                h_m = stage_m(order[idx], ah.pop(idx))
                stage_f(order[idx], h_m)
            elif seq == "MAF":
                h_m = stage_m(order[idx], ah.pop(idx))
                if nxt < n: ah[nxt] = run_a(nxt)
                stage_f(order[idx], h_m)
            elif seq == "MFA":
                h_m = stage_m(order[idx], ah.pop(idx))
                stage_f(order[idx], h_m)
                if nxt < n: ah[nxt] = run_a(nxt)

    nc.compile()
    return nc


def _prep(inputs):
    """Host-side preprocessing: fold LN gains into weights, compute effective
    biases, return (flags, extra per-core-constant input map)."""
    g1 = np.asarray(inputs["ln1_g"], np.float32)
    b1ln = np.asarray(inputs["ln1_b"], np.float32)
    g2 = np.asarray(inputs["ln2_g"], np.float32)
    b2ln = np.asarray(inputs["ln2_b"], np.float32)
    Wq = np.asarray(inputs["Wq"], np.float32).transpose(1, 0, 2).reshape(C, C)
    Wk = np.asarray(inputs["Wk"], np.float32).transpose(1, 0, 2).reshape(C, C)
    Wv = np.asarray(inputs["Wv"], np.float32).transpose(1, 0, 2).reshape(C, C)
    Wo = np.asarray(inputs["Wo"], np.float32)
    bo = np.asarray(inputs["bo"], np.float32)
    W1 = np.asarray(inputs["W1"], np.float32)
    b1 = np.asarray(inputs["b1"], np.float32)
    W2 = np.asarray(inputs["W2"], np.float32)
    b2 = np.asarray(inputs["b2"], np.float32)

    qb = b1ln @ Wq
    kb = b1ln @ Wk
    vb = b1ln @ Wv
    b1_eff = b1 + b2ln @ W1

    const = {
        "wq": g1[:, None] * Wq, "wk": g1[:, None] * Wk, "wv": g1[:, None] * Wv,
        "wo": Wo, "w1": g2[:, None] * W1, "w2": W2,
        "iden": np.eye(128, dtype=np.float32),
    }
    flags = (bool(np.any(qb)), bool(np.any(kb)), bool(np.any(vb)),
             bool(np.any(bo)), bool(np.any(b2)), bool(np.any(b1_eff)))
    if flags[0]: const["qb"] = qb
    if flags[1]: const["kb"] = kb
    if flags[2]: const["vb"] = vb
    if flags[3]: const["bo_r"] = bo
    if flags[4]: const["b2_r"] = b2
    if flags[5]: const["b1_r"] = b1_eff
    return flags, const


def kernel(**inputs):
    flags, const = _prep(inputs)
    if flags not in _PROGRAM_CACHE:
        _PROGRAM_CACHE[flags] = build_program(flags)
    nc = _PROGRAM_CACHE[flags]

    x = np.ascontiguousarray(np.asarray(inputs["x"], np.float32))
    in_maps = [dict(const, x=x[c * BC:(c + 1) * BC]) for c in range(N_CORES)]
    res = run_bass_kernel_spmd(nc, in_maps, core_ids=list(range(N_CORES)))
    return np.concatenate([res.results[c]["out"] for c in range(N_CORES)], axis=0)


if __name__ == "__main__":
    rng = np.random.default_rng(0)
    demo = {
        "x": rng.standard_normal((B, T, C), dtype=np.float32),
        "ln1_g": np.ones(C, np.float32), "ln1_b": np.zeros(C, np.float32),
        "Wq": rng.standard_normal((H, C, HD), dtype=np.float32) / np.sqrt(C),
        "Wk": rng.standard_normal((H, C, HD), dtype=np.float32) / np.sqrt(C),
        "Wv": rng.standard_normal((H, C, HD), dtype=np.float32) / np.sqrt(C),
        "Wo": rng.standard_normal((C, C), dtype=np.float32) / np.sqrt(C),
        "bo": np.zeros(C, np.float32),
        "ln2_g": np.ones(C, np.float32), "ln2_b": np.zeros(C, np.float32),
        "W1": rng.standard_normal((C, F), dtype=np.float32) / np.sqrt(C),
        "b1": np.zeros(F, np.float32),
        "W2": rng.standard_normal((F, C), dtype=np.float32) / np.sqrt(F),
        "b2": np.zeros(C, np.float32),
    }
    out = kernel(**demo)
    print("out", out.shape, out.dtype, float(np.abs(out).max()))



# revision 25
# speedup vs baseline: 1.0534x; 1.0534x over previous
"""Trainium2 Bass kernel for a dense transformer block (B=128, T=256, C=384,
H=6, HD=64, FFN=4C), data-parallel over batch across 8 NeuronCores.

Layout strategy (per core, 16 batch elements):
  - LayerNorm runs token-major ([128 tokens, 384] tiles, per-partition stats).
  - Matmul inputs are fp16 (PE runs 1 cycle/row at any free dim, FWL weight
    loads); accumulation is fp32 in PSUM; the residual stream stays fp32.
  - x1/x2/attn are transposed to feature-major with PE identity-matmuls so
    every matmul uses naturally-laid-out operands.
  - Softmax is computed s-major (scoresT = k_f^T q_f per head), exp on the
    scalar engine, causal mask applied by gpsimd affine_select (zero fill).
  - Attention output is computed token-major with the softmax denominator
    folded in as a 65th "ones" column of v; normalization is then a cheap
    per-partition reciprocal + broadcast multiply during PSUM evacuation.
  - LN gains are folded into the adjacent weight matrices host-side; biases
    (all zero in this problem) have exact fallback paths emitted only when
    nonzero at build time.
  - Scores (K=64 per head) pack head pairs into disjoint PE row groups via
    tile_position (0,0)/(64,0): the two matmuls run concurrently in the
    128x128 array, halving score time (invisible to CoreSim, real on HW).
  - fl5_head_ilv threads single-mp FFN1 filler matmuls between the i=0 attn
    head-pairs so the N=65 attn matmuls' LDWEIGHTS stream behind real work.
    Finer-grained filler variants (between i=1 pairs, inside transpose
    groups) were measured SLOWER on HW - the transpose->evac->consumer
    chains are latency-critical; don't put fillers inside them.
  - fp8e4 DoubleRow was evaluated and rejected: one fp8 GEMM alone costs
    ~2.2e-2 max-rel-err vs the 2e-2 gate (numpy study in fp8_study.py).
"""

import sys

sys.path.insert(0, "/opt/trn_rl_repo")

import numpy as np

import concourse.bass as bass
import concourse.tile as tile
from concourse import bacc, mybir
from concourse.bass_utils import run_bass_kernel_spmd

FP32 = mybir.dt.float32
FP16 = mybir.dt.float16
AF = mybir.ActivationFunctionType
ALU = mybir.AluOpType

N_CORES = 8
B, T, C, H, HD = 128, 256, 384, 6, 64
F = 4 * C  # 1536
BC = B // N_CORES  # 16 batches per core
NT = T // 128  # 2 token chunks per batch
NC_ = C // 128  # 3 feature chunks
NF = F // 128  # 12 hidden chunks
EPS = 1e-5
SCALE = HD ** -0.5

_PROGRAM_CACHE = {}
POOL_CFG = dict(apool=5, psA=6, psT=2, spool=8, lookahead=0, order="FLAG5",
                fl5_m4_early=True, fl3_relu_dve=2, fl5_f1_thirds=True,
                fl5_attn_split=True, fl5_splits=(2, 3, 5), fl5_f2_pos=2,
                fl5_head_ilv=True,
                rstd_pow=False, ln_apply_gpsimd=False, v_evac_scalar=False,
                head_pair=False, relu_dve=3, tr_evac_scalar="", qk_evac_dve=False,
                w384=True, fine_attn=False, attn_evac_act=False,
                xpool=3, xla=2, qk_merge=True, attn_inline=True,
                res16=False, attn_evac_act2=False,
                tsplit=True, qk_nosplit=True, ffn1_split=True,
                score_rowtile=True)


def soff_legacy(i, si):
    return {(0, 0): 0, (1, 0): 128, (1, 1): 256}[(i, si)]


def _patched_act_tables(arch):
    """Force every activation onto the one table set that contains all the
    functions this kernel uses (ln, exp, relu, copy, identity), so the ACT
    engine loads its spline tables exactly once instead of thrashing between
    per-function default sets (~1.3us per reload)."""
    import concourse.hw_specs as hw_specs
    full = hw_specs.get_activation_tables(arch)
    target = "natural_log_exp_and_others"
    return {k: (v if k == target else set()) for k, v in full.items()}


def build_program(flags, repeat=1):
    """flags: (use_qb, use_kb, use_vb, use_bo, use_b2, use_b1) booleans."""
    use_qb, use_kb, use_vb, use_bo, use_b2, use_b1 = flags
    bacc.get_activation_tables = _patched_act_tables
    nc = bacc.Bacc("TRN2", target_bir_lowering=False, debug=False,
                   num_devices=N_CORES)

    x_d = nc.dram_tensor("x", [BC, T, C], FP32, kind="ExternalInput").ap()
    wq_d = nc.dram_tensor("wq", [C, C], FP32, kind="ExternalInput").ap()
    wk_d = nc.dram_tensor("wk", [C, C], FP32, kind="ExternalInput").ap()
    wv_d = nc.dram_tensor("wv", [C, C], FP32, kind="ExternalInput").ap()
    wo_d = nc.dram_tensor("wo", [C, C], FP32, kind="ExternalInput").ap()
    w1_d = nc.dram_tensor("w1", [C, F], FP32, kind="ExternalInput").ap()
    w2_d = nc.dram_tensor("w2", [F, C], FP32, kind="ExternalInput").ap()
    id_d = nc.dram_tensor("iden", [128, 128], FP32, kind="ExternalInput").ap()
    qb_d = nc.dram_tensor("qb", [C], FP32, kind="ExternalInput").ap() if use_qb else None
    kb_d = nc.dram_tensor("kb", [C], FP32, kind="ExternalInput").ap() if use_kb else None
    vb_d = nc.dram_tensor("vb", [C], FP32, kind="ExternalInput").ap() if use_vb else None
    bo_d = nc.dram_tensor("bo_r", [C], FP32, kind="ExternalInput").ap() if use_bo else None
    b2_d = nc.dram_tensor("b2_r", [C], FP32, kind="ExternalInput").ap() if use_b2 else None
    b1_d = nc.dram_tensor("b1_r", [F], FP32, kind="ExternalInput").ap() if use_b1 else None
    out_d = nc.dram_tensor("out", [BC, T, C], FP32, kind="ExternalOutput").ap()

    from contextlib import ExitStack
    with tile.TileContext(nc) as tc, ExitStack() as ctx:
        wp = ctx.enter_context(tc.tile_pool(name="wpool", bufs=1))
        ap_ = ctx.enter_context(tc.tile_pool(name="apool", bufs=POOL_CFG["apool"]))
        sp = ctx.enter_context(tc.tile_pool(name="spool", bufs=POOL_CFG["spool"]))
        psA = ctx.enter_context(tc.tile_pool(name="psA", bufs=POOL_CFG["psA"], space="PSUM"))
        psT = ctx.enter_context(tc.tile_pool(name="psT", bufs=POOL_CFG["psT"], space="PSUM"))
        xp = ctx.enter_context(tc.tile_pool(name="xp", bufs=POOL_CFG.get("xpool", 3)))

        # ---- weights (fp16, cast during SWDGE DMA) ----
        wq_t = wp.tile([128, NC_, C], FP16, tag="wq")
        wk_t = wp.tile([128, NC_, C], FP16, tag="wk")
        wv_t = wp.tile([128, NC_, C], FP16, tag="wv")
        wo_t = wp.tile([128, NC_, C], FP16, tag="wo")
        w1_t = wp.tile([128, NC_, F], FP16, tag="w1")
        w2_t = wp.tile([128, NF, C], FP16, tag="w2")
        iden = wp.tile([128, 128], FP16, tag="iden")
        eps_t = wp.tile([128, 1], FP32, tag="eps")
        nc.vector.memset(eps_t, EPS)
        nc.gpsimd.dma_start(out=iden, in_=id_d)
        for wt, wd in ((wq_t, wq_d), (wk_t, wk_d), (wv_t, wv_d), (wo_t, wo_d)):
            nc.gpsimd.dma_start(out=wt, in_=wd.rearrange("(k p) n -> p k n", p=128))
        nc.gpsimd.dma_start(out=w1_t, in_=w1_d.rearrange("(k p) n -> p k n", p=128))
        nc.gpsimd.dma_start(out=w2_t, in_=w2_d.rearrange("(m p) n -> p m n", p=128))

        ones_row = None
        if use_vb or use_bo or use_b2:
            ones_row = wp.tile([1, 128], FP16, tag="ones_row")
            nc.vector.memset(ones_row, 1.0)
        qb_t = kb_t = None
        if use_qb:
            qb_t = wp.tile([128, NC_], FP32, tag="qb")
            nc.sync.dma_start(out=qb_t, in_=qb_d.rearrange("(m p) -> p m", p=128))
        if use_kb:
            kb_t = wp.tile([128, NC_], FP32, tag="kb")
            nc.sync.dma_start(out=kb_t, in_=kb_d.rearrange("(m p) -> p m", p=128))
        vb_t = bo_t = b2_t = b1_t = None
        if use_vb:
            vb_t = wp.tile([1, C], FP16, tag="vb")
            nc.gpsimd.dma_start(out=vb_t, in_=vb_d[None, :])
        if use_bo:
            bo_t = wp.tile([1, C], FP16, tag="bo")
            nc.gpsimd.dma_start(out=bo_t, in_=bo_d[None, :])
        if use_b2:
            b2_t = wp.tile([1, C], FP16, tag="b2")
            nc.gpsimd.dma_start(out=b2_t, in_=b2_d[None, :])
        if use_b1:
            b1_t = wp.tile([128, NF], FP32, tag="b1")
            nc.sync.dma_start(out=b1_t, in_=b1_d.rearrange("(m p) -> p m", p=128))

        def layer_norm(src, tag):
            mv = sp.tile([128, NT, 2], FP32, tag="mv")
            for i in range(NT):
                st = sp.tile([128, 6], FP32, tag="st")
                nc.vector.bn_stats(out=st, in_=src[:, i, :])
                nc.vector.bn_aggr(out=mv[:, i, :], in_=st)
            rstd = sp.tile([128, NT, 1], FP32, tag="rstd")
            if POOL_CFG.get("rstd_pow"):
                nc.gpsimd.tensor_scalar(
                    out=rstd, in0=mv[:, :, 1:2], scalar1=EPS, scalar2=-0.5,
                    op0=ALU.add, op1=ALU.pow)
            else:
                lnv = sp.tile([128, NT, 1], FP32, tag="lnv")
                nc.scalar.activation(out=lnv, in_=mv[:, :, 1:2], func=AF.Ln,
                                     bias=eps_t)
                nc.scalar.activation(out=rstd, in_=lnv, func=AF.Exp, scale=-0.5)
            dst = ap_.tile([128, NT, C], FP16, tag=tag)
            eng = nc.gpsimd if POOL_CFG.get("ln_apply_gpsimd") else nc.vector
            for i in range(NT):
                eng.tensor_scalar(
                    out=dst[:, i, :], in0=src[:, i, :],
                    scalar1=mv[:, i, 0:1], scalar2=rstd[:, i, :],
                    op0=ALU.subtract, op1=ALU.mult)
            return dst

        def stage_l(b):
            # ---- load x [128, 2, 384] (prefetched ahead of stage_a) ----
            if POOL_CFG.get("res16"):
                # fp16 residual stream: cast during DMA; halves DVE element
                # cost for LN stats/apply (2x 16-bit mode) + DMA bytes.
                x_t = xp.tile([128, NT, C], FP16, tag="x")
                eng = nc.gpsimd if POOL_CFG.get("res16_swdge") else nc.sync
                eng.dma_start(out=x_t,
                              in_=x_d[b].rearrange("(i p) c -> p i c", p=128))
            else:
                x_t = xp.tile([128, NT, C], FP32, tag="x")
                nc.sync.dma_start(out=x_t,
                                  in_=x_d[b].rearrange("(i p) c -> p i c", p=128))
            return x_t

        def stage_a(b, x_t):
            # ---- LN1 -> x1 fp16 token-major ----
            x1 = layer_norm(x_t, "x1")

            # ---- x1 -> feature-major x1f [128c, 3, 256t] ----
            x1f = ap_.tile([128, NC_, T], FP16, tag="x1f")
            x1f_evac = (nc.scalar.copy if "x1f" in POOL_CFG.get("tr_evac_scalar", "")
                        else nc.vector.tensor_copy)
            if POOL_CFG.get("tsplit"):
                # Per-t-half transpose banks: the copy for half 0 doesn't wait
                # on half 1's transposes (PSUM banks serialize at bank level).
                for i in range(NT):
                    p_tr = psT.tile([128, NC_, 128], FP16, tag="tr")
                    for j in range(NC_):
                        nc.tensor.transpose(p_tr[:, j, :],
                                            x1[:, i, 128 * j:128 * (j + 1)], iden)
                    x1f_evac(x1f[:, :, 128 * i:128 * (i + 1)], p_tr)
            else:
                p_tr = psT.tile([128, NC_, T], FP16, tag="tr")
                for i in range(NT):
                    for j in range(NC_):
                        nc.tensor.transpose(p_tr[:, j, 128 * i:128 * (i + 1)],
                                            x1[:, i, 128 * j:128 * (j + 1)], iden)
                x1f_evac(x1f, p_tr)

            # ---- q, k projections (feature-major out) ----
            if POOL_CFG.get("qk_merge") and qb_t is None and kb_t is None:
                # One [128, 2, T] PSUM bank per m-chunk holds q and k; one
                # ACT copy evacuates both. Halves psA pressure + evac count.
                qk_f = ap_.tile([128, 2, NC_, T], FP16, tag="qk_f")
                q_f = qk_f[:, 0]
                k_f = qk_f[:, 1]
                for m in range(NC_):
                    pq = psA.tile([128, 2, T], FP32, tag="a")
                    for qi, wt in ((0, wq_t), (1, wk_t)):
                        for kk in range(NC_):
                            nc.tensor.matmul(pq[:, qi, :],
                                             wt[:, kk, 128 * m:128 * (m + 1)],
                                             x1f[:, kk, :],
                                             start=(kk == 0), stop=(kk == NC_ - 1))
                    if POOL_CFG.get("qk_evac_dve"):
                        nc.vector.tensor_copy(qk_f[:, :, m, :], pq)
                    else:
                        nc.scalar.copy(out=qk_f[:, :, m, :], in_=pq)
            else:
                q_f = ap_.tile([128, NC_, T], FP16, tag="q_f")
                k_f = ap_.tile([128, NC_, T], FP16, tag="k_f")
                qk_halves = (((0, 128), (128, 128))
                             if POOL_CFG.get("tsplit") and not POOL_CFG.get("qk_nosplit")
                             else ((0, T),))
                for dst, wt, bias_t in ((q_f, wq_t, qb_t), (k_f, wk_t, kb_t)):
                    for m in range(NC_):
                        for lo, sz in qk_halves:
                            pq = psA.tile([128, T], FP32, tag="a")
                            for kk in range(NC_):
                                nc.tensor.matmul(pq[:, 0:sz],
                                                 wt[:, kk, 128 * m:128 * (m + 1)],
                                                 x1f[:, kk, lo:lo + sz],
                                                 start=(kk == 0), stop=(kk == NC_ - 1))
                            if bias_t is not None:
                                nc.scalar.activation(out=dst[:, m, lo:lo + sz],
                                                     in_=pq[:, 0:sz], func=AF.Identity,
                                                     bias=bias_t[:, m:m + 1])
                            elif POOL_CFG.get("qk_evac_dve"):
                                nc.vector.tensor_copy(dst[:, m, lo:lo + sz], pq[:, 0:sz])
                            else:
                                nc.scalar.copy(out=dst[:, m, lo:lo + sz], in_=pq[:, 0:sz])

            # ---- v projection (token-major, interleaved with ones col) ----
            v_t = ap_.tile([128, NT, H, HD + 1], FP16, tag="v_t")
            # ones column for the folded softmax denominator: one strided
            # gpsimd memset per batch, issued before the projections so it's
            # never on the critical path (and off the busy DVE queue).
            nc.gpsimd.memset(v_t[:, :, :, HD:HD + 1], 1.0)
            for i in range(NT):
                pv = psA.tile([128, C], FP32, tag="a")
                for kk in range(NC_):
                    nc.tensor.matmul(pv, x1f[:, kk, 128 * i:128 * (i + 1)],
                                     wv_t[:, kk, :], start=(kk == 0),
                                     stop=(kk == NC_ - 1 and vb_t is None))
                if vb_t is not None:
                    nc.tensor.matmul(pv, ones_row, vb_t, start=False, stop=True,
                                     skip_group_check=True)
                if POOL_CFG.get("v_evac_scalar"):
                    nc.scalar.copy(out=v_t[:, i, :, 0:HD],
                                   in_=pv.rearrange("p (h d) -> p h d", h=H))
                else:
                    nc.vector.tensor_copy(v_t[:, i, :, 0:HD],
                                          pv.rearrange("p (h d) -> p h d", h=H))

            # ---- scores (s-major), exp, causal mask ----
            if POOL_CFG.get("w384"):
                # Packed layout [128s, H, 384]: cols 0:256 are (si=0, t=0:256),
                # cols 256:384 are (si=1, t=128:256) — only the causally live
                # region is ever computed, exp'd, or masked.
                w_t = ap_.tile([128, H, 3 * 128], FP16, tag="w_t")
                for h in range(H):
                    j, r = h // 2, (h % 2) * 64
                    ps = psA.tile([128, 3 * 128], FP32, tag="a")
                    nc.tensor.matmul(ps[:, 0:T], k_f[r:r + 64, j, 0:128],
                                     q_f[r:r + 64, j, :], start=True, stop=True)
                    nc.tensor.matmul(ps[:, T:T + 128], k_f[r:r + 64, j, 128:T],
                                     q_f[r:r + 64, j, 128:T], start=True, stop=True)
                    nc.scalar.activation(out=w_t[:, h, :], in_=ps, func=AF.Exp,
                                         scale=SCALE)
                    for off in (0, 256):  # diagonal blocks: keep t-s >= 0
                        nc.gpsimd.affine_select(
                            out=w_t[:, h, off:off + 128],
                            in_=w_t[:, h, off:off + 128],
                            pattern=[[1, 128]], compare_op=ALU.is_ge,
                            fill=0.0, base=0, channel_multiplier=-1)
                return dict(x_t=x_t, w_t=w_t, v_t=v_t)
            w_t = ap_.tile([128, H, NT, T], FP16, tag="w_t")
            if POOL_CFG.get("head_pair"):
                for hp in range(H // 2):
                    ps = psA.tile([128, 2, NT, T], FP32, tag="a2")
                    for hh in range(2):
                        h = 2 * hp + hh
                        j, r = h // 2, (h % 2) * 64
                        nc.tensor.matmul(ps[:, hh, 0, :], k_f[r:r + 64, j, 0:128],
                                         q_f[r:r + 64, j, :], start=True, stop=True)
                        nc.tensor.matmul(ps[:, hh, 1, 128:T], k_f[r:r + 64, j, 128:T],
                                         q_f[r:r + 64, j, 128:T], start=True, stop=True)
                    nc.scalar.activation(out=w_t[:, 2 * hp:2 * hp + 2, :, :], in_=ps,
                                         func=AF.Exp, scale=SCALE)
                    nc.gpsimd.affine_select(
                        out=w_t[:, 2 * hp:2 * hp + 2, :, :],
                        in_=w_t[:, 2 * hp:2 * hp + 2, :, :],
                        pattern=[[0, 2], [-128, NT], [1, T]], compare_op=ALU.is_ge,
                        fill=0.0, base=0, channel_multiplier=-1)
            else:
                for h in range(H):
                    j, r = h // 2, (h % 2) * 64
                    ps = psA.tile([128, NT, T], FP32, tag="a")
                    nc.tensor.matmul(ps[:, 0, :], k_f[r:r + 64, j, 0:128],
                                     q_f[r:r + 64, j, :], start=True, stop=True)
                    nc.tensor.matmul(ps[:, 1, 128:T], k_f[r:r + 64, j, 128:T],
                                     q_f[r:r + 64, j, 128:T], start=True, stop=True)
                    nc.scalar.activation(out=w_t[:, h, :, :], in_=ps, func=AF.Exp,
                                         scale=SCALE)
                    nc.gpsimd.affine_select(
                        out=w_t[:, h, :, :], in_=w_t[:, h, :, :],
                        pattern=[[-128, NT], [1, T]], compare_op=ALU.is_ge,
                        fill=0.0, base=0, channel_multiplier=-1)

            return dict(x_t=x_t, w_t=w_t, v_t=v_t)

        def stage_m(b, hnd):
            x_t, w_t, v_t = hnd["x_t"], hnd["w_t"], hnd["v_t"]
            a_f = ap_.tile([128, NC_, T], FP16, tag="a_f")
            if POOL_CFG.get("fine_attn"):
                # Per (i, head-pair) processing: small PSUM tiles, reciprocal
                # and normalize per 128-col chunk, transpose immediately, and
                # per-i transpose banks so the a_f copy for t-chunk 0 doesn't
                # wait on t-chunk 1's transposes.
                soff = {(0, 0): 0, (1, 0): 128, (1, 1): 256}
                a_tok = ap_.tile([128, NT, C], FP16, tag="a_tok")
                for i in range(NT):
                    p_at = psT.tile([128, NC_, 128], FP16, tag="tr")
                    for j in range(NC_):
                        pa = psA.tile([128, 2, HD + 1], FP32, tag="a")
                        for hh in range(2):
                            h = 2 * j + hh
                            for si in range(i + 1):
                                o = soff[(i, si)]
                                nc.tensor.matmul(pa[:, hh, :],
                                                 w_t[:, h, o:o + 128],
                                                 v_t[:, si, h, :],
                                                 start=(si == 0), stop=(si == i))
                        r_t = sp.tile([128, 2, 1], FP32, tag="r_t")
                        nc.vector.reciprocal(r_t, pa[:, :, HD:HD + 1])
                        bcast = bass.AP(tensor=r_t.tensor, offset=r_t.offset,
                                        ap=[r_t.ap[0], [1, 2], [0, HD]])
                        nc.vector.tensor_tensor(
                            out=a_tok[:, i, 128 * j:128 * (j + 1)].rearrange(
                                "p (h d) -> p h d", h=2),
                            in0=pa[:, :, 0:HD], in1=bcast, op=ALU.mult)
                        nc.tensor.transpose(p_at[:, j, :],
                                            a_tok[:, i, 128 * j:128 * (j + 1)],
                                            iden)
                    if "a_f" in POOL_CFG.get("tr_evac_scalar", ""):
                        nc.scalar.copy(out=a_f[:, :, 128 * i:128 * (i + 1)],
                                       in_=p_at)
                    else:
                        nc.vector.tensor_copy(a_f[:, :, 128 * i:128 * (i + 1)],
                                              p_at)
            else:
                # ---- attention (token-major, Z in column 64) ----
                a_tok = ap_.tile([128, NT, C], FP16, tag="a_tok")
                a_f_evac = (nc.scalar.copy if "a_f" in POOL_CFG.get("tr_evac_scalar", "")
                            else nc.vector.tensor_copy)
                for i in range(NT):
                    pa = psA.tile([128, H, HD + 1], FP32, tag="a")
                    for h in range(H):
                        for si in range(i + 1):
                            w_src = (w_t[:, h, soff_legacy(i, si):soff_legacy(i, si) + 128]
                                     if POOL_CFG.get("w384") else
                                     w_t[:, h, si, 128 * i:128 * (i + 1)])
                            nc.tensor.matmul(pa[:, h, :], w_src,
                                             v_t[:, si, h, :],
                                             start=(si == 0), stop=(si == i))
                    if POOL_CFG.get("attn_evac_act") or POOL_CFG.get("attn_evac_act2"):
                        # Evacuate on ACT; normalize off-PSUM (gpsimd or
                        # all-fp16 DVE) to keep PSUM reads off the DVE path.
                        av = sp.tile([128, H, HD + 1], FP16, tag="av")
                        nc.scalar.copy(out=av, in_=pa)
                        r_t = sp.tile([128, H, 1], FP32, tag="r_t")
                        nc.vector.reciprocal(r_t, av[:, :, HD:HD + 1])
                        bcast = bass.AP(tensor=r_t.tensor, offset=r_t.offset,
                                        ap=[r_t.ap[0], [1, H], [0, HD]])
                        eng = (nc.vector if POOL_CFG.get("attn_evac_act2")
                               else nc.gpsimd)
                        eng.tensor_tensor(out=a_tok[:, i, :].rearrange(
                            "p (h d) -> p h d", h=H), in0=av[:, :, 0:HD],
                            in1=bcast, op=ALU.mult)
                    else:
                        r_t = sp.tile([128, H, 1], FP32, tag="r_t")
                        nc.vector.reciprocal(r_t, pa[:, :, HD:HD + 1])
                        bcast = bass.AP(tensor=r_t.tensor, offset=r_t.offset,
                                        ap=[r_t.ap[0], [1, H], [0, HD]])
                        nc.vector.tensor_tensor(out=a_tok[:, i, :].rearrange(
                            "p (h d) -> p h d", h=H), in0=pa[:, :, 0:HD], in1=bcast,
                            op=ALU.mult)
                    if POOL_CFG.get("tsplit") and POOL_CFG.get("attn_inline"):
                        # transpose+evacuate half i before issuing half i+1's
                        # matmuls: DVE order becomes recip0,tt0,copy0,recip1,…
                        p_at = psT.tile([128, NC_, 128], FP16, tag="tr")
                        for j in range(NC_):
                            nc.tensor.transpose(p_at[:, j, :],
                                                a_tok[:, i, 128 * j:128 * (j + 1)],
                                                iden)
                        a_f_evac(a_f[:, :, 128 * i:128 * (i + 1)], p_at)

                # ---- attn -> feature-major ----
                if POOL_CFG.get("tsplit") and POOL_CFG.get("attn_inline"):
                    pass  # done inline above
                elif POOL_CFG.get("tsplit"):
                    for i in range(NT):
                        p_at = psT.tile([128, NC_, 128], FP16, tag="tr")
                        for j in range(NC_):
                            nc.tensor.transpose(p_at[:, j, :],
                                                a_tok[:, i, 128 * j:128 * (j + 1)],
                                                iden)
                        a_f_evac(a_f[:, :, 128 * i:128 * (i + 1)], p_at)
                else:
                    p_at = psT.tile([128, NC_, T], FP16, tag="tr")
                    for i in range(NT):
                        for j in range(NC_):
                            nc.tensor.transpose(p_at[:, j, 128 * i:128 * (i + 1)],
                                                a_tok[:, i, 128 * j:128 * (j + 1)], iden)
                    a_f_evac(a_f, p_at)

            # ---- output projection + residual -> y ----
            y_t = ap_.tile([128, NT, C],
                           FP16 if POOL_CFG.get("res16") else FP32, tag="y_t")
            for i in range(NT):
                py = psA.tile([128, C], FP32, tag="a")
                for kk in range(NC_):
                    nc.tensor.matmul(py, a_f[:, kk, 128 * i:128 * (i + 1)],
                                     wo_t[:, kk, :], start=(kk == 0),
                                     stop=(kk == NC_ - 1 and bo_t is None))
                if bo_t is not None:
                    nc.tensor.matmul(py, ones_row, bo_t, start=False, stop=True,
                                     skip_group_check=True)
                nc.vector.tensor_tensor(out=y_t[:, i, :], in0=py, in1=x_t[:, i, :],
                                        op=ALU.add)

            # ---- LN2 -> x2 fp16 ----
            x2 = layer_norm(y_t, "x2")

            # ---- x2 -> feature-major ----
            x2f = ap_.tile([128, NC_, T], FP16, tag="x2f")
            x2f_evac = (nc.scalar.copy if "x2f" in POOL_CFG.get("tr_evac_scalar", "")
                        else nc.vector.tensor_copy)
            if POOL_CFG.get("tsplit"):
                for i in range(NT):
                    p_x2 = psT.tile([128, NC_, 128], FP16, tag="tr")
                    for j in range(NC_):
                        nc.tensor.transpose(p_x2[:, j, :],
                                            x2[:, i, 128 * j:128 * (j + 1)], iden)
                    x2f_evac(x2f[:, :, 128 * i:128 * (i + 1)], p_x2)
            else:
                p_x2 = psT.tile([128, NC_, T], FP16, tag="tr")
                for i in range(NT):
                    for j in range(NC_):
                        nc.tensor.transpose(p_x2[:, j, 128 * i:128 * (i + 1)],
                                            x2[:, i, 128 * j:128 * (j + 1)], iden)
                x2f_evac(x2f, p_x2)
            return dict(x2f=x2f, y_t=y_t)

        def stage_f(b, hnd):
            x2f, y_t = hnd["x2f"], hnd["y_t"]
            # ---- FFN1 + ReLU -> h_t fp16 (feature-major) ----
            h_t = ap_.tile([128, NF, T], FP16, tag="h_t")
            f1_halves = ((0, 128), (128, 128)) if POOL_CFG.get("ffn1_split") else ((0, T),)
            for mp in range(NF // 2):
                ph = psA.tile([128, 2, T], FP32, tag="a")
                for sub in range(2):
                    m = 2 * mp + sub
                    for lo, sz in f1_halves:
                        for kk in range(NC_):
                            nc.tensor.matmul(ph[:, sub, lo:lo + sz],
                                             w1_t[:, kk, 128 * m:128 * (m + 1)],
                                             x2f[:, kk, lo:lo + sz],
                                             start=(kk == 0), stop=(kk == NC_ - 1))
                if b1_t is not None:
                    for sub in range(2):
                        m = 2 * mp + sub
                        nc.vector.tensor_scalar(
                            out=h_t[:, m, :], in0=ph[:, sub, :],
                            scalar1=b1_t[:, m:m + 1], scalar2=0.0,
                            op0=ALU.add, op1=ALU.max)
                elif mp < POOL_CFG.get("relu_dve", 3):
                    nc.vector.tensor_scalar(
                        out=h_t[:, 2 * mp:2 * mp + 2, :], in0=ph,
                        scalar1=0.0, scalar2=None, op0=ALU.max)
                else:
                    nc.scalar.activation(out=h_t[:, 2 * mp:2 * mp + 2, :], in_=ph,
                                         func=AF.Relu)

            # ---- FFN2 + residual -> out ----
            res16 = POOL_CFG.get("res16")
            o_t = ap_.tile([128, NT, C], FP16 if res16 else FP32, tag="o_t")
            for i in range(NT):
                po = psA.tile([128, C], FP32, tag="a")
                for m in range(NF):
                    nc.tensor.matmul(po, h_t[:, m, 128 * i:128 * (i + 1)],
                                     w2_t[:, m, :], start=(m == 0),
                                     stop=(m == NF - 1 and b2_t is None))
                if b2_t is not None:
                    nc.tensor.matmul(po, ones_row, b2_t, start=False, stop=True,
                                     skip_group_check=True)
                nc.vector.tensor_tensor(out=o_t[:, i, :], in0=po, in1=y_t[:, i, :],
                                        op=ALU.add)

            out_ap = out_d[b].rearrange("(i p) c -> p i c", p=128)
            if res16 and POOL_CFG.get("res16_swdge"):
                nc.gpsimd.dma_start(out=out_ap, in_=o_t)  # cast fp16 -> fp32
            else:
                nc.sync.dma_start(out=out_ap, in_=o_t)

        # ===== FLAG3: sub-stage software pipeline ============================
        def fl3_a1(S):
            S["x1"] = layer_norm(S["x_t"], "x1")

        def fl3_a2(S, filler=None):
            x1 = S["x1"]
            x1f = ap_.tile([128, NC_, T], FP16, tag="x1f")
            if POOL_CFG.get("dma_tr_x"):
                # SBUF->SBUF transpose on the DMA xbar: no PE matmul, no PSUM
                # bank, no DVE evacuation copy.
                for i in range(NT):
                    for j in range(NC_):
                        nc.sync.dma_start_transpose(
                            x1f[:, j, 128 * i:128 * (i + 1)],
                            x1[:, i, 128 * j:128 * (j + 1)])
            else:
                for i in range(NT):
                    p_tr = psT.tile([128, NC_, 128], FP16, tag="tr")
                    for j in range(NC_):
                        nc.tensor.transpose(p_tr[:, j, :],
                                            x1[:, i, 128 * j:128 * (j + 1)], iden)
                    nc.vector.tensor_copy(x1f[:, :, 128 * i:128 * (i + 1)], p_tr)
                    if filler is not None and i == 0:
                        filler()
            S["x1f"] = x1f

        def fl3_a12(S):
            fl3_a1(S)
            fl3_a2(S)

        def fl3_a3(S):
            # q/k (merged psum) + v projections
            x1f = S["x1f"]
            qk_f = ap_.tile([128, 2, NC_, T], FP16, tag="qk_f")
            for m in range(NC_):
                pq = psA.tile([128, 2, T], FP32, tag="a")
                for qi, wt in ((0, wq_t), (1, wk_t)):
                    for kk in range(NC_):
                        nc.tensor.matmul(pq[:, qi, :],
                                         wt[:, kk, 128 * m:128 * (m + 1)],
                                         x1f[:, kk, :],
                                         start=(kk == 0), stop=(kk == NC_ - 1))
                if POOL_CFG.get("qk_evac_dve"):
                    nc.vector.tensor_copy(qk_f[:, :, m, :], pq)
                else:
                    nc.scalar.copy(out=qk_f[:, :, m, :], in_=pq)
            v_t = ap_.tile([128, NT, H, HD + 1], FP16, tag="v_t")
            nc.gpsimd.memset(v_t[:, :, :, HD:HD + 1], 1.0)
            for i in range(NT):
                pv = psA.tile([128, C], FP32, tag="a")
                for kk in range(NC_):
                    nc.tensor.matmul(pv, x1f[:, kk, 128 * i:128 * (i + 1)],
                                     wv_t[:, kk, :], start=(kk == 0),
                                     stop=(kk == NC_ - 1))
                if POOL_CFG.get("v_evac_scalar"):
                    nc.scalar.copy(out=v_t[:, i, :, 0:HD],
                                   in_=pv.rearrange("p (h d) -> p h d", h=H))
                else:
                    nc.vector.tensor_copy(v_t[:, i, :, 0:HD],
                                          pv.rearrange("p (h d) -> p h d", h=H))
            S["qk_f"], S["v_t"] = qk_f, v_t

        def fl3_scores_head(S, h, part=None):
            qk_f = S["qk_f"]
            q_f, k_f = qk_f[:, 0], qk_f[:, 1]
            w_t = S["w_t"]
            j, r = h // 2, (h % 2) * 64
            if part is None or part == 0:
                ps = psA.tile([128, 3 * 128], FP32, tag="a")
                S.setdefault("ps_h", {})[h] = ps
                nc.tensor.matmul(ps[:, 0:T], k_f[r:r + 64, j, 0:128],
                                 q_f[r:r + 64, j, :], start=True, stop=True)
                nc.tensor.matmul(ps[:, T:T + 128], k_f[r:r + 64, j, 128:T],
                                 q_f[r:r + 64, j, 128:T], start=True, stop=True)
            ps = S["ps_h"][h] if part is not None else ps
            if part is None:
                nc.scalar.activation(out=w_t[:, h, :], in_=ps, func=AF.Exp,
                                     scale=SCALE)
                offs = (0, 256)
            elif part == 0:
                # i=0-critical block only: attn(i=0) reads w_t[:, h, 0:128]
                nc.scalar.activation(out=w_t[:, h, 0:128], in_=ps[:, 0:128],
                                     func=AF.Exp, scale=SCALE)
                offs = (0,)
            else:
                nc.scalar.activation(out=w_t[:, h, 128:384], in_=ps[:, 128:384],
                                     func=AF.Exp, scale=SCALE)
                offs = (256,)
            for off in offs:
                nc.gpsimd.affine_select(
                    out=w_t[:, h, off:off + 128], in_=w_t[:, h, off:off + 128],
                    pattern=[[1, 128]], compare_op=ALU.is_ge,
                    fill=0.0, base=0, channel_multiplier=-1)

        def fl3_a4(S):
            # scores (packed w384), exp, diagonal-only causal mask
            w_t = ap_.tile([128, H, 3 * 128], FP16, tag="w_t")
            S["w_t"] = w_t
            if POOL_CFG.get("score_rowtile"):
                # K=64 per head: pack head pairs into disjoint PE row groups
                # (rows 0-63 / 64-127 via tile_position) so the two heads'
                # matmuls run concurrently in the array — halves score time.
                qk_f = S["qk_f"]
                q_f, k_f = qk_f[:, 0], qk_f[:, 1]
                for j in range(NC_):
                    ps_lo = psA.tile([128, 3 * 128], FP32, tag="a")
                    ps_hi = psA.tile([128, 3 * 128], FP32, tag="a")
                    pss = (ps_lo, ps_hi)
                    # (out_lo, out_sz, k_lo, q_lo): chunkA = s 0:128 x t 0:256;
                    # chunkB = s 128:256 x t 128:256 (packed at cols 256:384)
                    for out_lo, out_sz, k_lo, q_lo in ((0, T, 0, 0),
                                                       (T, 128, 128, 128)):
                        for r, ps in ((0, pss[0]), (64, pss[1])):
                            nc.tensor.matmul(
                                ps[:, out_lo:out_lo + out_sz],
                                k_f[r:r + 64, j, k_lo:k_lo + 128],
                                q_f[r:r + 64, j, q_lo:q_lo + out_sz],
                                start=True, stop=True, tile_position=(r, 0))
                    for hh, ps in enumerate(pss):
                        h = 2 * j + hh
                        nc.scalar.activation(out=w_t[:, h, :], in_=ps,
                                             func=AF.Exp, scale=SCALE)
                        for off in (0, 256):
                            nc.gpsimd.affine_select(
                                out=w_t[:, h, off:off + 128],
                                in_=w_t[:, h, off:off + 128],
                                pattern=[[1, 128]], compare_op=ALU.is_ge,
                                fill=0.0, base=0, channel_multiplier=-1)
                return
            if POOL_CFG.get("exp_split"):
                for h in range(H):
                    fl3_scores_head(S, h, part=0)
                for h in range(H):
                    fl3_scores_head(S, h, part=1)
                S.pop("ps_h")
            else:
                for h in range(H):
                    fl3_scores_head(S, h)

        def fl3_a34(S):
            # fused: per m-chunk, project q/k, evacuate, then immediately run
            # the two heads living in that chunk through scores+exp+mask —
            # head 0's exp no longer queues behind all three evacuations.
            x1f = S["x1f"]
            qk_f = ap_.tile([128, 2, NC_, T], FP16, tag="qk_f")
            S["qk_f"] = qk_f
            w_t = ap_.tile([128, H, 3 * 128], FP16, tag="w_t")
            S["w_t"] = w_t
            for m in range(NC_):
                pq = psA.tile([128, 2, T], FP32, tag="a")
                for qi, wt in ((0, wq_t), (1, wk_t)):
                    for kk in range(NC_):
                        nc.tensor.matmul(pq[:, qi, :],
                                         wt[:, kk, 128 * m:128 * (m + 1)],
                                         x1f[:, kk, :],
                                         start=(kk == 0), stop=(kk == NC_ - 1))
                nc.scalar.copy(out=qk_f[:, :, m, :], in_=pq)
                fl3_scores_head(S, 2 * m)
                fl3_scores_head(S, 2 * m + 1)
            v_t = ap_.tile([128, NT, H, HD + 1], FP16, tag="v_t")
            nc.gpsimd.memset(v_t[:, :, :, HD:HD + 1], 1.0)
            for i in range(NT):
                pv = psA.tile([128, C], FP32, tag="a")
                for kk in range(NC_):
                    nc.tensor.matmul(pv, x1f[:, kk, 128 * i:128 * (i + 1)],
                                     wv_t[:, kk, :], start=(kk == 0),
                                     stop=(kk == NC_ - 1))
                nc.vector.tensor_copy(v_t[:, i, :, 0:HD],
                                      pv.rearrange("p (h d) -> p h d", h=H))
            S["v_t"] = v_t

        def fl3_a34c(S):
            # pipelined qk/v/scores: evacuations overlap later projections and
            # each head-pair's exp issues as early as the data allows, so the
            # ACT queue drains w_t before the next iteration's attn matmuls.
            x1f = S["x1f"]
            qk_f = ap_.tile([128, 2, NC_, T], FP16, tag="qk_f")
            S["qk_f"] = qk_f
            w_t = ap_.tile([128, H, 3 * 128], FP16, tag="w_t")
            S["w_t"] = w_t
            q_f, k_f = qk_f[:, 0], qk_f[:, 1]
            v_t = ap_.tile([128, NT, H, HD + 1], FP16, tag="v_t")
            nc.gpsimd.memset(v_t[:, :, :, HD:HD + 1], 1.0)
            S["v_t"] = v_t

            def qk_mm(m):
                pq = psA.tile([128, 2, T], FP32, tag="a")
                for qi, wt in ((0, wq_t), (1, wk_t)):
                    for kk in range(NC_):
                        nc.tensor.matmul(pq[:, qi, :],
                                         wt[:, kk, 128 * m:128 * (m + 1)],
                                         x1f[:, kk, :],
                                         start=(kk == 0), stop=(kk == NC_ - 1))
                nc.scalar.copy(out=qk_f[:, :, m, :], in_=pq)

            def v_mm(i):
                pv = psA.tile([128, C], FP32, tag="a")
                for kk in range(NC_):
                    nc.tensor.matmul(pv, x1f[:, kk, 128 * i:128 * (i + 1)],
                                     wv_t[:, kk, :], start=(kk == 0),
                                     stop=(kk == NC_ - 1))
                nc.vector.tensor_copy(v_t[:, i, :, 0:HD],
                                      pv.rearrange("p (h d) -> p h d", h=H))

            def sc(j):
                ps_lo = psA.tile([128, 3 * 128], FP32, tag="a")
                ps_hi = psA.tile([128, 3 * 128], FP32, tag="a")
                for out_lo, out_sz, k_lo, q_lo in ((0, T, 0, 0),
                                                   (T, 128, 128, 128)):
                    for r, ps in ((0, ps_lo), (64, ps_hi)):
                        nc.tensor.matmul(
                            ps[:, out_lo:out_lo + out_sz],
                            k_f[r:r + 64, j, k_lo:k_lo + 128],
                            q_f[r:r + 64, j, q_lo:q_lo + out_sz],
                            start=True, stop=True, tile_position=(r, 0))
                for hh, ps in ((0, ps_lo), (1, ps_hi)):
                    h = 2 * j + hh
                    nc.scalar.activation(out=w_t[:, h, :], in_=ps,
                                         func=AF.Exp, scale=SCALE)
                    for off in (0, 256):
                        nc.gpsimd.affine_select(
                            out=w_t[:, h, off:off + 128],
                            in_=w_t[:, h, off:off + 128],
                            pattern=[[1, 128]], compare_op=ALU.is_ge,
                            fill=0.0, base=0, channel_multiplier=-1)

            qk_mm(0); qk_mm(1); v_mm(0); sc(0); qk_mm(2); v_mm(1); sc(1); sc(2)

        def fl3_mattn_mm_i(S, i, filler=None):
            w_t, v_t = S["w_t"], S["v_t"]
            soff = {(0, 0): 0, (1, 0): 128, (1, 1): 256}
            if i == 0:
                a_tok = ap_.tile([128, NT, C], FP16, tag="a_tok")
                S["a_tok"] = a_tok
            a_tok = S["a_tok"]
            pa = psA.tile([128, H, HD + 1], FP32, tag="a")
            for h in range(H):
                for si in range(i + 1):
                    o = soff[(i, si)]
                    nc.tensor.matmul(pa[:, h, :], w_t[:, h, o:o + 128],
                                     v_t[:, si, h, :],
                                     start=(si == 0), stop=(si == i))
                if filler is not None and (i == 1 or h % 2 == 1) and h < H - 1:
                    filler(h // 2)  # FFN filler between attn weight loads
            r_t = sp.tile([128, H, 1], FP32, tag="r_t")
            nc.vector.reciprocal(r_t, pa[:, :, HD:HD + 1])
            bcast = bass.AP(tensor=r_t.tensor, offset=r_t.offset,
                            ap=[r_t.ap[0], [1, H], [0, HD]])
            nc.vector.tensor_tensor(out=a_tok[:, i, :].rearrange(
                "p (h d) -> p h d", h=H), in0=pa[:, :, 0:HD], in1=bcast,
                op=ALU.mult)

        def fl3_mattn_mm(S):
            for i in range(NT):
                fl3_mattn_mm_i(S, i)

        def fl3_mattn_tr(S, filler=None):
            a_tok = S["a_tok"]
            a_f = ap_.tile([128, NC_, T], FP16, tag="a_f")
            if POOL_CFG.get("dma_tr_a"):
                for i in range(NT):
                    for j in range(NC_):
                        nc.sync.dma_start_transpose(
                            a_f[:, j, 128 * i:128 * (i + 1)],
                            a_tok[:, i, 128 * j:128 * (j + 1)])
            else:
                for i in range(NT):
                    p_at = psT.tile([128, NC_, 128], FP16, tag="tr")
                    for j in range(NC_):
                        nc.tensor.transpose(p_at[:, j, :],
                                            a_tok[:, i, 128 * j:128 * (j + 1)], iden)
                    nc.vector.tensor_copy(a_f[:, :, 128 * i:128 * (i + 1)], p_at)
                    if filler is not None and i == 0:
                        filler()
            S["a_f"] = a_f

        def fl3_mattn(S):
            fl3_mattn_mm(S)
            fl3_mattn_tr(S)

        def fl3_m3(S):
            # Wo projection + residual + LN2
            a_f, x_t = S["a_f"], S["x_t"]
            y_t = ap_.tile([128, NT, C], FP32, tag="y_t")
            for i in range(NT):
                py = psA.tile([128, C], FP32, tag="a")
                for kk in range(NC_):
                    nc.tensor.matmul(py, a_f[:, kk, 128 * i:128 * (i + 1)],
                                     wo_t[:, kk, :], start=(kk == 0),
                                     stop=(kk == NC_ - 1))
                nc.vector.tensor_tensor(out=y_t[:, i, :], in0=py,
                                        in1=x_t[:, i, :], op=ALU.add)
            S["y_t"] = y_t
            S["x2"] = layer_norm(y_t, "x2")

        def fl3_m4(S, filler=None):
            x2 = S["x2"]
            x2f = ap_.tile([128, NC_, T], FP16, tag="x2f")
            if POOL_CFG.get("dma_tr_x"):
                for i in range(NT):
                    for j in range(NC_):
                        nc.sync.dma_start_transpose(
                            x2f[:, j, 128 * i:128 * (i + 1)],
                            x2[:, i, 128 * j:128 * (j + 1)])
            else:
                for i in range(NT):
                    p_x2 = psT.tile([128, NC_, 128], FP16, tag="tr")
                    for j in range(NC_):
                        nc.tensor.transpose(p_x2[:, j, :],
                                            x2[:, i, 128 * j:128 * (j + 1)], iden)
                    nc.vector.tensor_copy(x2f[:, :, 128 * i:128 * (i + 1)], p_x2)
                    if filler is not None and i == 0:
                        filler()
            S["x2f"] = x2f

        def fl3_f1(S, mp_lo=0, mp_hi=NF // 2):
            # FFN1 + ReLU (ACT so LN1(k+1) can start on DVE underneath)
            x2f = S["x2f"]
            if mp_lo == 0:
                h_t = ap_.tile([128, NF, T], FP16, tag="h_t")
                S["h_t"] = h_t
            h_t = S["h_t"]
            for mp in range(mp_lo, mp_hi):
                ph = psA.tile([128, 2, T], FP32, tag="a")
                for sub in range(2):
                    m = 2 * mp + sub
                    for kk in range(NC_):
                        nc.tensor.matmul(ph[:, sub, :],
                                         w1_t[:, kk, 128 * m:128 * (m + 1)],
                                         x2f[:, kk, :],
                                         start=(kk == 0), stop=(kk == NC_ - 1))
                if mp < POOL_CFG.get("fl3_relu_dve", 0):
                    nc.vector.tensor_scalar(
                        out=h_t[:, 2 * mp:2 * mp + 2, :], in0=ph,
                        scalar1=0.0, scalar2=None, op0=ALU.max)
                else:
                    nc.scalar.activation(out=h_t[:, 2 * mp:2 * mp + 2, :],
                                         in_=ph, func=AF.Relu)

        def fl3_f1m(S, m):
            # single-m FFN1 unit (3 matmuls N=256 + one ReLU evac): the
            # filler currency for hiding small-MM weight loads elsewhere.
            x2f = S["x2f"]
            if m == 0:
                h_t = ap_.tile([128, NF, T], FP16, tag="h_t")
                S["h_t"] = h_t
            h_t = S["h_t"]
            ph1 = psA.tile([128, T], FP32, tag="a")
            for kk in range(NC_):
                nc.tensor.matmul(ph1, w1_t[:, kk, 128 * m:128 * (m + 1)],
                                 x2f[:, kk, :], start=(kk == 0),
                                 stop=(kk == NC_ - 1))
            if (m // 2) < POOL_CFG.get("fl3_relu_dve", 0):
                nc.vector.tensor_scalar(out=h_t[:, m, :], in0=ph1,
                                        scalar1=0.0, scalar2=None, op0=ALU.max)
            else:
                nc.scalar.activation(out=h_t[:, m, :], in_=ph1, func=AF.Relu)

        def fl3_f2(S):
            h_t, y_t = S["h_t"], S["y_t"]
            o_t = ap_.tile([128, NT, C], FP32, tag="o_t")
            for i in range(NT):
                po = psA.tile([128, C], FP32, tag="a")
                for m in range(NF):
                    nc.tensor.matmul(po, h_t[:, m, 128 * i:128 * (i + 1)],
                                     w2_t[:, m, :], start=(m == 0),
                                     stop=(m == NF - 1))
                nc.vector.tensor_tensor(out=o_t[:, i, :], in0=po,
                                        in1=y_t[:, i, :], op=ALU.add)
            nc.sync.dma_start(out=out_d[S["b"]].rearrange("(i p) c -> p i c", p=128),
                              in_=o_t)

        order = [bb for _ in range(repeat) for bb in range(BC)]
        n = len(order)
        la = POOL_CFG.get("lookahead", 1)
        xla = POOL_CFG.get("xla", 2)
        seq = POOL_CFG.get("order", "AMF")
        xh = {}

        def ensure_x(upto):
            for q in range(min(upto + 1, n)):
                if q not in xh:
                    xh[q] = stage_l(order[q])

        def run_a(j):
            ensure_x(j + xla)
            return stage_a(order[j], xh.pop(j))

        if seq in ("FLAG3", "FLAG4", "FLAG5", "FLAG6"):
            SS = {}

            def getS(j):
                if j not in SS:
                    ensure_x(j + xla)
                    SS[j] = {"b": order[j], "x_t": xh.pop(j)}
                return SS[j]

            if seq == "FLAG6":
                # pair-stepped FLAG5: two batches per pipeline slot so each
                # cross-engine sync latency is amortized over 2x PE filler.
                assert n % 2 == 0
                HALF = NF // 4
                for j in (0, 1):
                    fl3_a12(getS(j)); fl3_a3(getS(j)); fl3_a4(getS(j))
                for k in range(0, n, 2):
                    ks = (k, k + 1)
                    nxt = [j for j in (k + 2, k + 3) if j < n]
                    prv = [j for j in (k - 2, k - 1) if j >= 0]
                    for j in nxt:
                        fl3_a1(getS(j))
                    for j in prv:
                        fl3_f1(SS[j], 0, HALF)
                    for j in ks:
                        fl3_mattn_mm(SS[j])
                    for j in prv:
                        fl3_f1(SS[j], HALF, NF // 2)
                    for j in ks:
                        fl3_mattn_tr(SS[j])
                    for j in nxt:
                        fl3_a2(SS[j])
                    for j in ks:
                        fl3_m3(SS[j])
                    for j in prv:
                        fl3_f2(SS.pop(j))
                    for j in ks:
                        fl3_m4(SS[j])
                    for j in nxt:
                        fl3_a3(SS[j])
                    for j in nxt:
                        fl3_a4(SS[j])
                for j in (n - 2, n - 1):
                    fl3_f1(SS[j])
                for j in (n - 2, n - 1):
                    fl3_f2(SS.pop(j))
                seq = "DONE"

            # prologue: batch 0 fully through stage-a
            if seq != "DONE":
                fl3_a12(getS(0)); fl3_a3(getS(0)); fl3_a4(getS(0))
            if seq == "FLAG5":
                HALF = NF // 4  # FFN1 pair-split point
                a2_early = POOL_CFG.get("fl5_a2_early")
                m4_early = POOL_CFG.get("fl5_m4_early")
                three = POOL_CFG.get("fl5_f1_thirds")
                ln_deep = POOL_CFG.get("fl5_ln_deep")
                if ln_deep and n > 1:
                    fl3_a1(getS(1))
                for k in range(n):
                    if k + 1 < n:
                        fl3_a1(getS(k + 2 if ln_deep else k + 1)) \
                            if (not ln_deep or k + 2 < n) else None
                    s1, s2, s3 = POOL_CFG.get("fl5_splits", (2, 3, 4))
                    if POOL_CFG.get("fl5_head_ilv2"):
                        # head_ilv variant: same coarse mp-granular fillers,
                        # but one unit threaded between each transpose pair
                        # (mattn_tr / a2) to hide those LDWEIGHTS too.
                        prev = SS[k - 1] if k - 1 >= 0 else None
                        mp_st = {"mp": 0}

                        def fmp(_h=None):
                            if prev is not None and mp_st["mp"] < NF // 2:
                                fl3_f1(prev, mp_st["mp"], mp_st["mp"] + 1)
                                mp_st["mp"] += 1
                        fl3_mattn_mm_i(SS[k], 0, filler=fmp)
                        fmp()
                        fl3_mattn_mm_i(SS[k], 1)
                        fmp()
                        fl3_mattn_tr(SS[k], filler=fmp)
                        if k + 1 < n:
                            fl3_a2(SS[k + 1], filler=fmp)
                        fl3_m3(SS[k])
                        fl3_m4(SS[k])
                        while prev is not None and mp_st["mp"] < NF // 2:
                            fmp()
                        if k + 1 < n:
                            fl3_a3(SS[k + 1])
                            fl3_a4(SS[k + 1])
                        if prev is not None:
                            fl3_f2(SS.pop(k - 1))
                        continue
                    if POOL_CFG.get("f1m_ilv"):
                        # m-granular FFN1 fillers woven between every small-N
                        # PE instruction group (attn heads, transpose halves)
                        # so their LDWEIGHTS stream behind real matmul work.
                        prev = SS[k - 1] if k - 1 >= 0 else None
                        mst = {"m": 0}

                        def fm(_h=None, n_units=1):
                            if prev is None:
                                return
                            while n_units > 0 and mst["m"] < NF:
                                fl3_f1m(prev, mst["m"])
                                mst["m"] += 1
                                n_units -= 1
                        fl3_mattn_mm_i(SS[k], 0, filler=fm)
                        fm()
                        fl3_mattn_mm_i(SS[k], 1, filler=fm)
                        fm()
                        fl3_mattn_tr(SS[k], filler=fm)
                        if k + 1 < n:
                            fl3_a2(SS[k + 1], filler=fm)
                        fl3_m3(SS[k])
                        fl3_m4(SS[k], filler=fm)
                        fm(n_units=NF)  # drain whatever's left
                        if k + 1 < n:
                            fl3_a3(SS[k + 1])
                            fl3_a4(SS[k + 1])
                        if prev is not None:
                            fl3_f2(SS.pop(k - 1))
                        continue
                    if (POOL_CFG.get("fl5_hilv_i1") and k - 1 >= 0
                            and POOL_CFG.get("fl5_attn_split")):
                        # head_ilv + fillers between i=1 attn head-pairs too
                        # (use fl5_splits=(2,3,6) so the post-tr f1 is empty)
                        prev = SS[k - 1]
                        fl3_mattn_mm_i(
                            SS[k], 0,
                            filler=lambda jj, p=prev: fl3_f1(p, jj, jj + 1))
                        fl3_f1(prev, 2, 3)
                        i1_cnt = {"i": 0}

                        def i1f(jj, p=prev):
                            i1_cnt["i"] += 1
                            if i1_cnt["i"] in (2, 4):  # after heads 1 and 3
                                m = 2 + i1_cnt["i"] // 2  # mp 3, then 4
                                fl3_f1(p, m, m + 1)
                        fl3_mattn_mm_i(SS[k], 1, filler=i1f)
                        fl3_f1(prev, 5, 6)
                    elif (POOL_CFG.get("fl5_head_ilv") and k - 1 >= 0
                            and POOL_CFG.get("fl5_attn_split")):
                        prev = SS[k - 1]
                        fl3_mattn_mm_i(
                            SS[k], 0,
                            filler=lambda jj, p=prev: fl3_f1(p, jj, jj + 1))
                        fl3_f1(prev, 2, s2)
                        fl3_mattn_mm_i(SS[k], 1)
                        fl3_f1(prev, s2, s3)
                    elif POOL_CFG.get("fl5_attn_split") and k - 1 >= 0:
                        fl3_f1(SS[k - 1], 0, s1)
                        fl3_mattn_mm_i(SS[k], 0)
                        fl3_f1(SS[k - 1], s1, s2)
                        fl3_mattn_mm_i(SS[k], 1)
                        fl3_f1(SS[k - 1], s2, s3)
                    else:
                        if k - 1 >= 0:
                            fl3_f1(SS[k - 1], 0, 2 if three else HALF)
                        fl3_mattn_mm(SS[k])
                        if k - 1 >= 0:
                            fl3_f1(SS[k - 1], 2 if three else HALF,
                                   4 if three else NF // 2)
                    if a2_early and k + 1 < n:
                        fl3_a2(SS[k + 1])
                    fl3_mattn_tr(SS[k])
                    if k - 1 >= 0:
                        if POOL_CFG.get("fl5_attn_split"):
                            fl3_f1(SS[k - 1], s3, NF // 2)
                        elif three:
                            fl3_f1(SS[k - 1], 4, NF // 2)
                    if not a2_early and k + 1 < n:
                        fl3_a2(SS[k + 1])
                    fl3_m3(SS[k])
                    f2_pos = POOL_CFG.get("fl5_f2_pos", 0)
                    if k - 1 >= 0 and f2_pos == 0:
                        fl3_f2(SS.pop(k - 1))
                    if POOL_CFG.get("fl5_fuse34"):
                        fl3_m4(SS[k])
                        if k + 1 < n:
                            fl3_a34(SS[k + 1])
                    elif m4_early:
                        fl3_m4(SS[k])
                        if k + 1 < n:
                            if POOL_CFG.get("a34c"):
                                fl3_a34c(SS[k + 1])
                            else:
                                fl3_a3(SS[k + 1])
                        if k - 1 >= 0 and f2_pos == 1:
                            fl3_f2(SS.pop(k - 1))
                        if k + 1 < n and not POOL_CFG.get("a34c"):
                            fl3_a4(SS[k + 1])
                        if k - 1 >= 0 and f2_pos == 2:
                            fl3_f2(SS.pop(k - 1))
                    else:
                        if k + 1 < n:
                            fl3_a3(SS[k + 1])
                        fl3_m4(SS[k])
                        if k + 1 < n:
                            fl3_a4(SS[k + 1])
                fl3_f1(SS[n - 1]); fl3_f2(SS.pop(n - 1))
            elif seq == "FLAG4":
                for k in range(n):
                    if k + 1 < n:
                        fl3_a12(getS(k + 1))
                    if k - 1 >= 0:
                        fl3_f1(SS[k - 1])
                    fl3_mattn(SS[k])
                    fl3_m3(SS[k])
                    if k - 1 >= 0:
                        fl3_f2(SS.pop(k - 1))
                    if k + 1 < n:
                        fl3_a3(SS[k + 1])
                    fl3_m4(SS[k])
                    if k + 1 < n:
                        fl3_a4(SS[k + 1])
                fl3_f1(SS[n - 1]); fl3_f2(SS.pop(n - 1))
            elif seq == "FLAG3":
                for k in range(n):
                    fl3_mattn(SS[k])
                    fl3_m3(SS[k])
                    if k - 1 >= 0:
                        fl3_f1(SS[k - 1])
                    fl3_m4(SS[k])
                    if k + 1 < n:
                        fl3_a12(getS(k + 1))
                    if k - 1 >= 0:
                        fl3_f2(SS.pop(k - 1))
                    if k + 1 < n:
                        fl3_a3(SS[k + 1])
                        fl3_a4(SS[k + 1])
                fl3_f1(SS[n - 1]); fl3_f2(SS.pop(n - 1))
            seq = "DONE"

        ah = {}
        for j in range(min(la + 1, n) if seq != "DONE" else 0):
            ah[j] = run_a(j)
        if seq == "FLAG1":
            mh = {}
            for idx in range(n):
                nxt = idx + la + 1
                if nxt < n: ah[nxt] = run_a(nxt)
                mh[idx] = stage_m(order[idx], ah.pop(idx))
                if idx - 1 in mh:
                    stage_f(order[idx - 1], mh.pop(idx - 1))
            stage_f(order[n - 1], mh.pop(n - 1))
        for idx in (range(n) if seq not in ("FLAG1", "DONE") else []):
            nxt = idx + la + 1
            if seq == "AMF":
                if nxt < n: ah[nxt] = run_a(nxt)
                h_m = stage_m(order[idx], ah.pop(idx))
                stage_f(order[idx], h_m)
            elif seq == "MAF":
                h_m = stage_m(order[idx], ah.pop(idx))
                if nxt < n: ah[nxt] = run_a(nxt)
                stage_f(order[idx], h_m)
            elif seq == "MFA":
                h_m = stage_m(order[idx], ah.pop(idx))
                stage_f(order[idx], h_m)
                if nxt < n: ah[nxt] = run_a(nxt)

    nc.compile()
    return nc


def _prep(inputs):
    """Host-side preprocessing: fold LN gains into weights, compute effective
    biases, return (flags, extra per-core-constant input map)."""
    g1 = np.asarray(inputs["ln1_g"], np.float32)
    b1ln = np.asarray(inputs["ln1_b"], np.float32)
    g2 = np.asarray(inputs["ln2_g"], np.float32)
    b2ln = np.asarray(inputs["ln2_b"], np.float32)
    Wq = np.asarray(inputs["Wq"], np.float32).transpose(1, 0, 2).reshape(C, C)
    Wk = np.asarray(inputs["Wk"], np.float32).transpose(1, 0, 2).reshape(C, C)
    Wv = np.asarray(inputs["Wv"], np.float32).transpose(1, 0, 2).reshape(C, C)
    Wo = np.asarray(inputs["Wo"], np.float32)
    bo = np.asarray(inputs["bo"], np.float32)
    W1 = np.asarray(inputs["W1"], np.float32)
    b1 = np.asarray(inputs["b1"], np.float32)
    W2 = np.asarray(inputs["W2"], np.float32)
    b2 = np.asarray(inputs["b2"], np.float32)

    qb = b1ln @ Wq
    kb = b1ln @ Wk
    vb = b1ln @ Wv
    b1_eff = b1 + b2ln @ W1

    const = {
        "wq": g1[:, None] * Wq, "wk": g1[:, None] * Wk, "wv": g1[:, None] * Wv,
        "wo": Wo, "w1": g2[:, None] * W1, "w2": W2,
        "iden": np.eye(128, dtype=np.float32),
    }
    flags = (bool(np.any(qb)), bool(np.any(kb)), bool(np.any(vb)),
             bool(np.any(bo)), bool(np.any(b2)), bool(np.any(b1_eff)))
    if flags[0]: const["qb"] = qb
    if flags[1]: const["kb"] = kb
    if flags[2]: const["vb"] = vb
    if flags[3]: const["bo_r"] = bo
    if flags[4]: const["b2_r"] = b2
    if flags[5]: const["b1_r"] = b1_eff
    return flags, const


def kernel(**inputs):
    flags, const = _prep(inputs)
    if flags not in _PROGRAM_CACHE:
        _PROGRAM_CACHE[flags] = build_program(flags)
    nc = _PROGRAM_CACHE[flags]

    x = np.ascontiguousarray(np.asarray(inputs["x"], np.float32))
    in_maps = [dict(const, x=x[c * BC:(c + 1) * BC]) for c in range(N_CORES)]
    res = run_bass_kernel_spmd(nc, in_maps, core_ids=list(range(N_CORES)))
    return np.concatenate([res.results[c]["out"] for c in range(N_CORES)], axis=0)


if __name__ == "__main__":
    rng = np.random.default_rng(0)
    demo = {
        "x": rng.standard_normal((B, T, C), dtype=np.float32),
        "ln1_g": np.ones(C, np.float32), "ln1_b": np.zeros(C, np.float32),
        "Wq": rng.standard_normal((H, C, HD), dtype=np.float32) / np.sqrt(C),
        "Wk": rng.standard_normal((H, C, HD), dtype=np.float32) / np.sqrt(C),
        "Wv": rng.standard_normal((H, C, HD), dtype=np.float32) / np.sqrt(C),
        "Wo": rng.standard_normal((C, C), dtype=np.float32) / np.sqrt(C),
        "bo": np.zeros(C, np.float32),
        "ln2_g": np.ones(C, np.float32), "ln2_b": np.zeros(C, np.float32),
        "W1": rng.standard_normal((C, F), dtype=np.float32) / np.sqrt(C),
        "b1": np.zeros(F, np.float32),
        "W2": rng.standard_normal((F, C), dtype=np.float32) / np.sqrt(F),
        "b2": np.zeros(C, np.float32),
    }
    out = kernel(**demo)
    print("out", out.shape, out.dtype, float(np.abs(out).max()))

